# revision 1
# baseline (speedup 1.0000x reference)
"""Trainium2 Bass kernel for the combined loss (KL + CE + InfoNCE + focal + adv CE).

Strategy (8 NeuronCores, data-parallel over the batch):
  - The three [4096, 1000] tensors (output / master_net_pred / output_adv) and the
    targets are sharded by rows: 512 rows per core.
  - InfoNCE: feats = concat(feat_pooled, feat_pooled_masked) -> [8192, 256].
    Every core receives the FULL feature matrix, transposed to [256, 8192] and
    with its columns rolled (per core) so that the core's own 1024 query rows sit
    at fixed column positions {0..512, 4096..4608}.  This makes the single
    compiled program core-independent: each core normalizes the full matrix
    on-device, computes its 1024x8192 stripe of the cosine-similarity Gram
    matrix on the PE, and does a streaming (no-max, bounded-logit) logsumexp
    via the ScalarEngine's fused exp+row-accumulate.  The diagonal is masked by
    accumulating -1e9*I into the PSUM group via an extra tiny matmul; positives
    are the diagonal of the block 4096 columns away (extracted with an
    identity-mask tensor_tensor_reduce).
  - Each core emits 5 partial sums; the host sums the 8x[8] outputs and applies
    the loss weights.
"""

import numpy as np

import concourse.bacc as bacc
import concourse.tile as tile
from concourse import mybir
from concourse.bass_utils import run_bass_kernel_spmd

F32 = mybir.dt.float32
BF16 = mybir.dt.bfloat16
AF = mybir.ActivationFunctionType
ALU = mybir.AluOpType
AX = mybir.AxisListType

NCORES = 8
B, C, D = 4096, 1000, 256
RB = B // NCORES          # 512 rows of the [B, C] tensors per core
NT = RB // 128            # 4 row-tiles per core
N2 = 2 * B                # 8192 infoNCE rows
CH = 2048                 # column chunk for the Gram stripe
NCH = N2 // CH            # 4 chunks
SUB = 512                 # matmul moving free dim (fp32 max)
NSUB = CH // SUB
# q-block column starts after the per-core column roll (identical on all cores)
QCOLS = [0, 128, 256, 384, N2 // 2, N2 // 2 + 128, N2 // 2 + 256, N2 // 2 + 384]
QB = len(QCOLS)

KL_TEMP = 4.0
KL_INTERP = 0.5
NCE_TEMP = 0.07
NEG_BIG = -1.0e9


def _build_module(phases=("norm", "nce", "cekl")):
    phases = set(phases)
    nc = bacc.Bacc("TRN2", target_bir_lowering=False, debug=False)

    o_d = nc.dram_tensor("o", [RB, C], F32, kind="ExternalInput")
    m_d = nc.dram_tensor("m", [RB, C], F32, kind="ExternalInput")
    a_d = nc.dram_tensor("a", [RB, C], F32, kind="ExternalInput")
    tg_d = nc.dram_tensor("tg", [128, NT], F32, kind="ExternalInput")
    ta_d = nc.dram_tensor("ta", [128, NT], F32, kind="ExternalInput")
    ft_d = nc.dram_tensor("ft", [256, N2], F32, kind="ExternalInput")
    res_d = nc.dram_tensor("res", [8, 1], F32, kind="ExternalOutput")

    import ml_dtypes
    iota_np = np.tile(np.arange(C, dtype=np.float32), (128, 1))
    ident_np = np.eye(128, dtype=np.float32)
    identb_np = np.eye(128).astype(ml_dtypes.bfloat16)
    negidb_np = (NEG_BIG * np.eye(128)).astype(ml_dtypes.bfloat16)
    ones_np = np.ones((128, 128), dtype=np.float32)
    iota_d = nc.inline_tensor(iota_np, "iota_c")
    ident_d = nc.inline_tensor(ident_np, "ident_c")
    identb_d = nc.inline_tensor(identb_np, "identb_c")
    negidb_d = nc.inline_tensor(negidb_np, "negidb_c")
    ones_d = nc.inline_tensor(ones_np, "ones_c")

    from contextlib import ExitStack
    with tile.TileContext(nc) as tc:
        with (
            tc.tile_pool(name="persist", bufs=1) as persist,
            tc.tile_pool(name="bigscr", bufs=3) as bigp,
            tc.tile_pool(name="ps", bufs=2, space="PSUM") as psp,
            ExitStack() as late_ctx,
        ):
            dma = nc.default_dma_engine.dma_start

            iota_t = persist.tile([128, C], F32, tag="iota")
            dma(out=iota_t[:], in_=iota_d[:])
            ident_t = persist.tile([128, 128], F32, tag="ident")
            dma(out=ident_t[:], in_=ident_d[:])
            identb_t = persist.tile([128, 128], BF16, tag="identb")
            dma(out=identb_t[:], in_=identb_d[:])
            negidb_t = persist.tile([128, 128], BF16, tag="negidb")
            dma(out=negidb_t[:], in_=negidb_d[:])
            ones_t = persist.tile([128, 128], F32, tag="ones")
            dma(out=ones_t[:], in_=ones_d[:])
            tg_t = persist.tile([128, NT], F32, tag="tg")
            dma(out=tg_t[:], in_=tg_d[:])
            ta_t = persist.tile([128, NT], F32, tag="ta")
            dma(out=ta_t[:], in_=ta_d[:])

            raw_pool_cm = tc.tile_pool(name="raw", bufs=1)
            rawp = raw_pool_cm.__enter__()
            h0 = rawp.tile([128, N2], F32, tag="h0")
            dma(out=h0[:], in_=ft_d[0:128, :])
            h1 = rawp.tile([128, N2], F32, tag="h1")
            dma(out=h1[:], in_=ft_d[128:256, :])

            # ---- normalize the full feature matrix into bf16 copies ----
            # rn = 1/||col||: column sums of squares via ones-matmul (result
            # broadcast across all 128 partitions), then exp(-0.5*ln(s)).
            # (DVE reciprocal measures ~10us/2k-tile on HW; Ln+Exp stay on ACT
            # and share one activation table with the big Exp passes.)
            rn = rawp.tile([128, N2], F32, tag="rn")
            for j in range(NCH if "norm" in phases else 0):
                ps_n = psp.tile([128, CH], F32, tag="ps")
                s0 = bigp.tile([128, CH], F32, tag="bigscr")
                nc.scalar.square(s0[:], h0[:, j * CH:(j + 1) * CH])
                s1 = bigp.tile([128, CH], F32, tag="bigscr")
                nc.scalar.square(s1[:], h1[:, j * CH:(j + 1) * CH])
                for s in range(NSUB):
                    sl = slice(s * SUB, (s + 1) * SUB)
                    nc.tensor.matmul(ps_n[:, sl], ones_t[:], s0[:, sl],
                                     start=True, stop=False)
                    nc.tensor.matmul(ps_n[:, sl], ones_t[:], s1[:, sl],
                                     start=False, stop=True)
                nc.scalar.activation(rn[:, j * CH:(j + 1) * CH], ps_n[:], AF.Ln)
            h0b = persist.tile([128, N2], BF16, tag="h0b")
            h1b = persist.tile([128, N2], BF16, tag="h1b")
            if "norm" in phases:
                nc.scalar.activation(rn[:], rn[:], AF.Exp, scale=-0.5)
                nc.vector.tensor_mul(h0b[:], h0[:], rn[:])
                nc.vector.tensor_mul(h1b[:], h1[:], rn[:])
            else:
                nc.vector.tensor_copy(h0b[:], h0[:])
                nc.vector.tensor_copy(h1b[:], h1[:])
            # raw fp32 features + rn are dead now — release their SBUF before
            # the io/scr/vec pools open.
            raw_pool_cm.__exit__(None, None, None)
            scrp = late_ctx.enter_context(tc.tile_pool(name="scr", bufs=4))
            iop = late_ctx.enter_context(tc.tile_pool(name="io", bufs=2))
            vecp = late_ctx.enter_context(tc.tile_pool(name="vec", bufs=1))

            # ---- InfoNCE stripe: 8 q-blocks x full 8192 columns ----
            rs_all = vecp.tile([128, QB], F32, tag="rs_all")    # sum exp(logits)
            posx_all = vecp.tile([128, QB], F32, tag="posx")    # exp(pos logit)
            if "nce" not in phases:
                nc.vector.memset(rs_all[:], 1.0)
                nc.vector.memset(posx_all[:], 1.0)
            for qi, q0 in enumerate(QCOLS if "nce" in phases else []):
                p0 = (q0 + N2 // 2) % N2
                lhsT0 = h0b[:, q0:q0 + 128]
                lhsT1 = h1b[:, q0:q0 + 128]
                rsp = scrp.tile([128, NCH], F32, tag="rsp")
                for chx in range(NCH):
                    base = chx * CH
                    ps_t = psp.tile([128, CH], F32, tag="ps")
                    for s in range(NSUB):
                        c0 = base + s * SUB
                        sl = slice(s * SUB, (s + 1) * SUB)
                        nc.tensor.matmul(ps_t[:, sl], lhsT0, h0b[:, c0:c0 + SUB],
                                         start=True, stop=False)
                        if c0 <= q0 < c0 + SUB:
                            off = s * SUB + (q0 - c0)
                            nc.tensor.matmul(ps_t[:, off:off + 128], negidb_t[:],
                                             identb_t[:], start=False, stop=False,
                                             skip_group_check=True)
                        nc.tensor.matmul(ps_t[:, sl], lhsT1, h1b[:, c0:c0 + SUB],
                                         start=False, stop=True)
                    es = bigp.tile([128, CH], F32, tag="bigscr")
                    nc.scalar.activation(es[:], ps_t[:], AF.Exp,
                                         scale=float(1.0 / NCE_TEMP),
                                         accum_out=rsp[:, chx:chx + 1])
                    if base <= p0 < base + CH:
                        # NB: tensor_tensor_reduce faults this device — use
                        # scalar_tensor_tensor to pick exp(pos) off the SBUF
                        # exp tile via the identity mask.
                        poff = p0 - base
                        pscr = scrp.tile([128, 128], F32, tag="posscr")
                        nc.vector.scalar_tensor_tensor(
                            out=pscr[:], in0=es[:, poff:poff + 128],
                            scalar=1.0, in1=ident_t[:],
                            op0=ALU.mult, op1=ALU.mult,
                            accum_out=posx_all[:, qi:qi + 1])
                nc.vector.reduce_sum(rs_all[:, qi:qi + 1], rsp[:], axis=AX.X)

            # ---- per-row CE / KL / focal / adv stats ----
            S1 = vecp.tile([128, NT], F32, tag="S1")   # sum exp(o)
            ST = vecp.tile([128, NT], F32, tag="ST")   # sum exp(o/T)
            SM = vecp.tile([128, NT], F32, tag="SM")   # sum exp(m/T)
            SA = vecp.tile([128, NT], F32, tag="SA")   # sum exp(a)
            PP = vecp.tile([128, NT], F32, tag="PP")   # sum exp(m/T)*(m-o)
            GO = vecp.tile([128, NT], F32, tag="GO")   # o[target]
            GA = vecp.tile([128, NT], F32, tag="GA")   # a[target_adv]
            if "cekl" not in phases:
                for st in (S1, ST, SM, SA):
                    nc.vector.memset(st[:], 1.0)
                nc.vector.memset(PP[:], 0.0)
                nc.vector.memset(GO[:], -1.0)   # keeps pt<1 so Ln(1-pt) finite
                nc.vector.memset(GA[:], 0.0)
            for t in range(NT if "cekl" in phases else 0):
                rsl = slice(t * 128, (t + 1) * 128)
                o_t = iop.tile([128, C], F32, tag="o")
                dma(out=o_t[:], in_=o_d[rsl, :])
                m_t = iop.tile([128, C], F32, tag="m")
                dma(out=m_t[:], in_=m_d[rsl, :])
                a_t = iop.tile([128, C], F32, tag="a")
                dma(out=a_t[:], in_=a_d[rsl, :])

                e1 = scrp.tile([128, C], F32, tag="scr1000")
                nc.scalar.activation(e1[:], o_t[:], AF.Exp, scale=1.0,
                                     accum_out=S1[:, t:t + 1])
                e2 = scrp.tile([128, C], F32, tag="scr1000")
                nc.scalar.activation(e2[:], o_t[:], AF.Exp,
                                     scale=float(1.0 / KL_TEMP),
                                     accum_out=ST[:, t:t + 1])
                em_t = iop.tile([128, C], F32, tag="em")
                nc.scalar.activation(em_t[:], m_t[:], AF.Exp,
                                     scale=float(1.0 / KL_TEMP),
                                     accum_out=SM[:, t:t + 1])
                e3 = scrp.tile([128, C], F32, tag="scr1000")
                nc.scalar.activation(e3[:], a_t[:], AF.Exp, scale=1.0,
                                     accum_out=SA[:, t:t + 1])

                d_t = iop.tile([128, C], F32, tag="d")
                nc.vector.tensor_sub(d_t[:], m_t[:], o_t[:])
                pr = scrp.tile([128, C], F32, tag="scr1000")
                nc.vector.scalar_tensor_tensor(
                    out=pr[:], in0=d_t[:], scalar=1.0, in1=em_t[:],
                    op0=ALU.mult, op1=ALU.mult, accum_out=PP[:, t:t + 1])
                g1 = scrp.tile([128, C], F32, tag="scr1000")
                nc.vector.scalar_tensor_tensor(
                    out=g1[:], in0=iota_t[:], scalar=tg_t[:, t:t + 1],
                    in1=o_t[:], op0=ALU.is_equal, op1=ALU.mult,
                    accum_out=GO[:, t:t + 1])
                g2 = scrp.tile([128, C], F32, tag="scr1000")
                nc.vector.scalar_tensor_tensor(
                    out=g2[:], in0=iota_t[:], scalar=ta_t[:, t:t + 1],
                    in1=a_t[:], op0=ALU.is_equal, op1=ALU.mult,
                    accum_out=GA[:, t:t + 1])

            # ---- epilogue on [128, NT] / [128, QB] stat vectors ----
            lse1 = vecp.tile([128, NT], F32, tag="lse1")
            nc.scalar.activation(lse1[:], S1[:], AF.Ln)
            lseT = vecp.tile([128, NT], F32, tag="lseT")
            nc.scalar.activation(lseT[:], ST[:], AF.Ln)
            lsem = vecp.tile([128, NT], F32, tag="lsem")
            nc.scalar.activation(lsem[:], SM[:], AF.Ln)
            lsea = vecp.tile([128, NT], F32, tag="lsea")
            nc.scalar.activation(lsea[:], SA[:], AF.Ln)

            ce = vecp.tile([128, NT], F32, tag="ce")      # per-row CE (positive)
            nc.vector.tensor_sub(ce[:], lse1[:], GO[:])
            adv = vecp.tile([128, NT], F32, tag="adv")
            nc.vector.tensor_sub(adv[:], lsea[:], GA[:])

            # kl_row = PP/(T*SM) - lsem + lseT
            invSM = vecp.tile([128, NT], F32, tag="invSM")
            nc.vector.reciprocal(invSM[:], SM[:])
            kl = vecp.tile([128, NT], F32, tag="kl")
            nc.vector.tensor_mul(kl[:], PP[:], invSM[:])
            nc.vector.tensor_scalar_mul(kl[:], kl[:], float(1.0 / KL_TEMP))
            nc.vector.tensor_sub(kl[:], kl[:], lsem[:])
            nc.vector.tensor_add(kl[:], kl[:], lseT[:])

            # focal_row = (1-pt)^gamma * ce,  pt = exp(-ce)
            pt = vecp.tile([128, NT], F32, tag="pt")
            nc.scalar.activation(pt[:], ce[:], AF.Exp, scale=-1.0)
            c1 = vecp.tile([128, NT], F32, tag="c1")
            nc.vector.tensor_scalar(c1[:], pt[:], 0.5, None, op0=ALU.is_lt)
            c2 = vecp.tile([128, NT], F32, tag="c2")
            nc.vector.tensor_scalar(c2[:], pt[:], 0.2, None, op0=ALU.is_lt)
            gam = vecp.tile([128, NT], F32, tag="gam")
            nc.vector.tensor_add(gam[:], c1[:], c2[:])
            nc.vector.tensor_scalar(gam[:], gam[:], 2.0, 1.0,
                                    op0=ALU.mult, op1=ALU.add)
            u = vecp.tile([128, NT], F32, tag="u")       # 1 - pt
            nc.vector.tensor_scalar(u[:], pt[:], -1.0, 1.0,
                                    op0=ALU.mult, op1=ALU.add)
            lg = vecp.tile([128, NT], F32, tag="lg")
            nc.scalar.activation(lg[:], u[:], AF.Ln)
            w = vecp.tile([128, NT], F32, tag="w")
            nc.vector.tensor_mul(w[:], gam[:], lg[:])
            nc.scalar.activation(w[:], w[:], AF.Exp)     # (1-pt)^gamma
            foc = vecp.tile([128, NT], F32, tag="foc")
            nc.vector.tensor_mul(foc[:], w[:], ce[:])

            # nce_row = ln(rowsum) - ln(exp(pos))
            lsen = vecp.tile([128, QB], F32, tag="lsen")
            nc.scalar.activation(lsen[:], rs_all[:], AF.Ln)
            posl = vecp.tile([128, QB], F32, tag="posl")
            nc.scalar.activation(posl[:], posx_all[:], AF.Ln)
            nce = vecp.tile([128, QB], F32, tag="nce")
            nc.vector.tensor_sub(nce[:], lsen[:], posl[:])

            # ---- reduce to 5 partial sums, then across partitions via PE ----
            acc = vecp.tile([128, 8], F32, tag="acc")
            nc.vector.reduce_sum(acc[:, 0:1], kl[:], axis=AX.X)
            nc.vector.reduce_sum(acc[:, 1:2], ce[:], axis=AX.X)
            nc.vector.reduce_sum(acc[:, 2:3], adv[:], axis=AX.X)
            nc.vector.reduce_sum(acc[:, 3:4], foc[:], axis=AX.X)
            nc.vector.reduce_sum(acc[:, 4:5], nce[:], axis=AX.X)
            nc.vector.memset(acc[:, 5:8], 0.0)

            ps_f = psp.tile([8, 1], F32, tag="ps")
            nc.tensor.matmul(ps_f[:], acc[:], ones_t[:, 0:1],
                             start=True, stop=True)
            out_sb = vecp.tile([8, 1], F32, tag="out_sb")
            nc.scalar.copy(out_sb[:], ps_f[:])
            dma(out=res_d[:], in_=out_sb[:])

    nc.compile()
    return nc


_NC = None


def _get_nc():
    global _NC
    if _NC is None:
        _NC = _build_module()
    return _NC


def _prep_inputs(output, target, master_net_pred, feat_pooled,
                 feat_pooled_masked, output_adv, target_adv):
    o = np.ascontiguousarray(np.asarray(output, dtype=np.float32))
    m = np.ascontiguousarray(np.asarray(master_net_pred, dtype=np.float32))
    a = np.ascontiguousarray(np.asarray(output_adv, dtype=np.float32))
    tg = np.asarray(target).astype(np.int64)
    ta = np.asarray(target_adv).astype(np.int64)
    f0 = np.asarray(feat_pooled, dtype=np.float32)
    f1 = np.asarray(feat_pooled_masked, dtype=np.float32)
    feats = np.concatenate([f0, f1], axis=0)  # [2B, D]

    in_maps = []
    for cc in range(NCORES):
        sl = slice(cc * RB, (cc + 1) * RB)
        roll = np.concatenate([np.arange(cc * RB, B), np.arange(0, cc * RB)])
        order = np.concatenate([roll, B + roll])
        ftc = np.ascontiguousarray(feats[order].T)  # [D, 2B], cols rolled
        in_maps.append({
            "o": o[sl].copy(),
            "m": m[sl].copy(),
            "a": a[sl].copy(),
            "tg": np.ascontiguousarray(
                tg[sl].reshape(NT, 128).T.astype(np.float32)),
            "ta": np.ascontiguousarray(
                ta[sl].reshape(NT, 128).T.astype(np.float32)),
            "ft": ftc,
        })
    return in_maps


def _combine(results):
    r = np.zeros(8, dtype=np.float64)
    for rr in results:
        r += rr["res"].reshape(-1).astype(np.float64)
    kl_mean = r[0] / (B * C)
    ce_mean = r[1] / B
    adv_mean = r[2] / B
    foc_mean = r[3] / B
    nce_mean = r[4] / (2 * B)
    loss = (KL_INTERP * KL_TEMP * KL_TEMP) * kl_mean \
        + (1.0 - KL_INTERP) * ce_mean + nce_mean + foc_mean + adv_mean
    return np.asarray([loss], dtype=np.float32)


def kernel(**inputs):
    in_maps = _prep_inputs(**inputs)
    out = run_bass_kernel_spmd(_get_nc(), in_maps,
                               core_ids=list(range(NCORES)))
    return _combine(out.results)


if __name__ == "__main__":
    rng = np.random.default_rng(0)
    ins = {
        "output": rng.standard_normal((B, C), dtype=np.float32),
        "target": rng.integers(0, C, size=(B,)),
        "master_net_pred": rng.standard_normal((B, C), dtype=np.float32),
        "feat_pooled": rng.standard_normal((B, D), dtype=np.float32),
        "feat_pooled_masked": rng.standard_normal((B, D), dtype=np.float32),
        "output_adv": rng.standard_normal((B, C), dtype=np.float32),
        "target_adv": rng.integers(0, C, size=(B,)),
    }
    print(kernel(**ins))



# revision 7
# speedup vs baseline: 1.4553x; 1.4553x over previous
"""Trainium2 Bass kernel for the combined loss (KL + CE + InfoNCE + focal + adv CE).

Strategy (8 NeuronCores, data-parallel over the batch):
  - The three [4096, 1000] tensors (output / master_net_pred / output_adv) are
    sharded by rows (512 rows per core) and shipped as bf16 packed [128, 4*1000]
    (partition p, tile t holds row t*128+p).  Targets ship as f32 [128, 4].
  - InfoNCE: feats = concat(feat_pooled, feat_pooled_masked) -> [8192, 256].
    Every core receives the FULL feature matrix (bf16), transposed to
    [256, 8192] and with its columns rolled per core so its own 1024 query rows
    sit at columns {0..512, 4096..4608} -- one compiled program for all cores.
  - On-device pipeline (engines balanced so the ScalarEngine exp stream is the
    only long pole):
      * column norms: DVE squares (bf16 2x) -> bf16 ones-matmul partition sums
        -> per-chunk Ln -> per-chunk Exp(-0.5*ln) -> DVE normalize muls.  No
        fp32 matmuls, no ACT squares (vs the old version).
      * CE/KL/adv exps run batched as single [128, 4000] activations in the
        initial DMA window; per-row sums via DVE reduces; gathers via
        iota==target masks with DVE accumulate.
      * InfoNCE: per 128-row q-block, 4 x [128, 2048] PSUM Gram chunks (bf16
        matmuls, 1024-col moving operands), fused exp(1/T * x) with row-sum
        accumulate; self-diagonal masked by accumulating -1e9*I via a tiny
        extra matmul; positives read off the exp tile with an identity mask.
  - Each core emits 5 partial sums; the host sums the 8x[8] outputs and applies
    the loss weights.
"""

import numpy as np

import concourse.bacc as bacc
import concourse.tile as tile
from concourse import mybir
from concourse.bass_utils import run_bass_kernel_spmd

F32 = mybir.dt.float32
BF16 = mybir.dt.bfloat16
AF = mybir.ActivationFunctionType
ALU = mybir.AluOpType
AX = mybir.AxisListType

NCORES = 8
B, C, D = 4096, 1000, 256
RB = B // NCORES          # 512 rows of the [B, C] tensors per core
NT = RB // 128            # 4 row-tiles per core
N2 = 2 * B                # 8192 infoNCE rows
CH = 2048                 # column chunk for the Gram stripe
NCH = N2 // CH            # 4 chunks
SUB = 512                 # matmul moving free dim (PSUM bank limit)
NSUB = CH // SUB
# q-block column starts after the per-core column roll (identical on all cores)
QCOLS = [0, 128, 256, 384, N2 // 2, N2 // 2 + 128, N2 // 2 + 256, N2 // 2 + 384]
QB = len(QCOLS)

KL_TEMP = 4.0
KL_INTERP = 0.5
NCE_TEMP = 0.07
NEG_BIG = -1.0e9


def _build_module():
    nc = bacc.Bacc("TRN2", target_bir_lowering=False, debug=False)

    o_d = nc.dram_tensor("o", [128, NT * C], BF16, kind="ExternalInput")
    m_d = nc.dram_tensor("m", [128, NT * C], BF16, kind="ExternalInput")
    a_d = nc.dram_tensor("a", [128, NT * C], BF16, kind="ExternalInput")
    tg_d = nc.dram_tensor("tg", [128, NT], F32, kind="ExternalInput")
    ta_d = nc.dram_tensor("ta", [128, NT], F32, kind="ExternalInput")
    f0_d = nc.dram_tensor("f0", [128, N2], BF16, kind="ExternalInput")
    f1_d = nc.dram_tensor("f1", [128, N2], BF16, kind="ExternalInput")
    res_d = nc.dram_tensor("res", [8, 1], F32, kind="ExternalOutput")

    import ml_dtypes
    iota_np = np.tile(np.arange(C, dtype=np.float32), (128, 1))
    ident_np = np.eye(128, dtype=np.float32)
    identb_np = np.eye(128).astype(ml_dtypes.bfloat16)
    negidb_np = (NEG_BIG * np.eye(128)).astype(ml_dtypes.bfloat16)
    onesb_np = np.ones((128, 128)).astype(ml_dtypes.bfloat16)
    iota_d = nc.inline_tensor(iota_np, "iota_c")
    ident_d = nc.inline_tensor(ident_np, "ident_c")
    identb_d = nc.inline_tensor(identb_np, "identb_c")
    negidb_d = nc.inline_tensor(negidb_np, "negidb_c")
    onesb_d = nc.inline_tensor(onesb_np, "onesb_c")

    from contextlib import ExitStack
    with tile.TileContext(nc) as tc:
        with (
            tc.tile_pool(name="persist", bufs=1) as persist,
            tc.tile_pool(name="scr", bufs=4) as scrp,
            tc.tile_pool(name="cescr", bufs=3) as cescr,
            tc.tile_pool(name="vec", bufs=1) as vecp,
            tc.tile_pool(name="ps", bufs=2, space="PSUM") as psp,
            ExitStack() as late_ctx,
        ):
            dma = nc.default_dma_engine.dma_start

            # ---- constants (small, issue first) ----
            iota_t = persist.tile([128, C], F32, tag="iota")
            dma(out=iota_t[:], in_=iota_d[:])
            ident_t = persist.tile([128, 128], F32, tag="ident")
            dma(out=ident_t[:], in_=ident_d[:])
            identb_t = persist.tile([128, 128], BF16, tag="identb")
            dma(out=identb_t[:], in_=identb_d[:])
            negidb_t = persist.tile([128, 128], BF16, tag="negidb")
            dma(out=negidb_t[:], in_=negidb_d[:])
            onesb_t = persist.tile([128, 128], BF16, tag="onesb")
            dma(out=onesb_t[:], in_=onesb_d[:])
            tg_t = persist.tile([128, NT], F32, tag="tg")
            dma(out=tg_t[:], in_=tg_d[:])
            ta_t = persist.tile([128, NT], F32, tag="ta")
            dma(out=ta_t[:], in_=ta_d[:])

            # ---- big inputs: interleave features (chunked) with logits ----
            raw_pool_cm = tc.tile_pool(name="raw", bufs=1)
            rawp = raw_pool_cm.__enter__()
            h0 = rawp.tile([128, N2], BF16, tag="h0")
            h1 = rawp.tile([128, N2], BF16, tag="h1")
            h0b = persist.tile([128, N2], BF16, tag="h0b")
            h1b = persist.tile([128, N2], BF16, tag="h1b")
            o_t = persist.tile([128, NT * C], BF16, tag="o")
            m_t = persist.tile([128, NT * C], BF16, tag="m")
            a_t = persist.tile([128, NT * C], BF16, tag="a")

            def dmac(j):
                sl = slice(j * CH, (j + 1) * CH)
                dma(out=h0[:, sl], in_=f0_d[:, sl])
                dma(out=h1[:, sl], in_=f1_d[:, sl])

            dmac(0)
            dma(out=o_t[:], in_=o_d[:])
            dmac(1)
            dma(out=m_t[:], in_=m_d[:])
            dmac(2)
            dmac(3)
            dma(out=a_t[:], in_=a_d[:])

            # ---- stat tiles ----
            st4 = vecp.tile([128, 16], F32, tag="st4")   # S1 | ST | SM | SA
            gst = vecp.tile([128, 12], F32, tag="gst")   # GO | GA | PP
            rsq = vecp.tile([128, 16], F32, tag="rsq")   # rs_all(8) | posx(8)

            # ---- norm phase: rn = exp(-0.5*ln(colsum(h^2))) chunk-pipelined.
            # squares on DVE (bf16 2x), partition sums via bf16 ones-matmul,
            # Ln straight off PSUM, Exp and normalize muls per chunk so the
            # Gram can start as soon as chunk 0 is normalized.
            def norm_chunk(j):
                sl = slice(j * CH, (j + 1) * CH)
                s0 = scrp.tile([128, CH], BF16, tag="sq")
                nc.vector.tensor_mul(s0[:], h0[:, sl], h0[:, sl])
                s1 = scrp.tile([128, CH], BF16, tag="sq")
                nc.vector.tensor_mul(s1[:], h1[:, sl], h1[:, sl])
                ps_n = psp.tile([128, CH], F32, tag="ps")
                for s in range(NSUB):
                    ssl = slice(s * SUB, (s + 1) * SUB)
                    nc.tensor.matmul(ps_n[:, ssl], onesb_t[:], s0[:, ssl],
                                     start=True, stop=False)
                    nc.tensor.matmul(ps_n[:, ssl], onesb_t[:], s1[:, ssl],
                                     start=False, stop=True)
                rl = scrp.tile([128, CH], BF16, tag="rnl")
                nc.scalar.activation(rl[:], ps_n[:], AF.Ln)
                rnc = scrp.tile([128, CH], BF16, tag="rnc")
                nc.scalar.activation(rnc[:], rl[:], AF.Exp, scale=-0.5)
                nc.vector.tensor_mul(h0b[:, sl], h0[:, sl], rnc[:])
                nc.vector.tensor_mul(h1b[:, sl], h1[:, sl], rnc[:])

            # norm chunks interleaved with the batched cekl exps so ACT stays
            # fed during the initial DMA window.
            norm_chunk(0)
            e1 = cescr.tile([128, NT * C], BF16, tag="ce")
            nc.scalar.activation(e1[:], o_t[:], AF.Exp, scale=1.0)
            eT = cescr.tile([128, NT * C], BF16, tag="ce")
            nc.scalar.activation(eT[:], o_t[:], AF.Exp,
                                 scale=float(1.0 / KL_TEMP))
            norm_chunk(1)
            em = persist.tile([128, NT * C], BF16, tag="em")
            nc.scalar.activation(em[:], m_t[:], AF.Exp,
                                 scale=float(1.0 / KL_TEMP))
            norm_chunk(2)
            norm_chunk(3)
            ea = cescr.tile([128, NT * C], BF16, tag="ce")
            nc.scalar.activation(ea[:], a_t[:], AF.Exp, scale=1.0)

            # raw features are dead once normalized -- release their SBUF
            # before the NCE exp-tile pool opens.
            raw_pool_cm.__exit__(None, None, None)
            esp = late_ctx.enter_context(tc.tile_pool(name="esp", bufs=3))

            # ---- cekl DVE chain (fills DVE while ACT grinds the Gram exps) ----
            for t in range(NT):
                csl = slice(t * C, (t + 1) * C)
                nc.vector.reduce_sum(st4[:, t:t + 1], e1[:, csl], axis=AX.X)
                nc.vector.reduce_sum(st4[:, 4 + t:5 + t], eT[:, csl], axis=AX.X)
                nc.vector.reduce_sum(st4[:, 8 + t:9 + t], em[:, csl], axis=AX.X)
            d_t = cescr.tile([128, NT * C], BF16, tag="d")
            nc.vector.tensor_sub(d_t[:], m_t[:], o_t[:])
            for t in range(NT):
                csl = slice(t * C, (t + 1) * C)
                pr = scrp.tile([128, C], BF16, tag="pr")
                nc.vector.scalar_tensor_tensor(
                    out=pr[:], in0=d_t[:, csl], scalar=1.0, in1=em[:, csl],
                    op0=ALU.mult, op1=ALU.mult, accum_out=gst[:, 8 + t:9 + t])
                g1 = scrp.tile([128, C], BF16, tag="pr")
                nc.vector.scalar_tensor_tensor(
                    out=g1[:], in0=iota_t[:], scalar=tg_t[:, t:t + 1],
                    in1=o_t[:, csl], op0=ALU.is_equal, op1=ALU.mult,
                    accum_out=gst[:, t:t + 1])
                g2 = scrp.tile([128, C], BF16, tag="pr")
                nc.vector.scalar_tensor_tensor(
                    out=g2[:], in0=iota_t[:], scalar=ta_t[:, t:t + 1],
                    in1=a_t[:, csl], op0=ALU.is_equal, op1=ALU.mult,
                    accum_out=gst[:, 4 + t:5 + t])
                nc.vector.reduce_sum(st4[:, 12 + t:13 + t], ea[:, csl],
                                     axis=AX.X)

            # ---- InfoNCE stripe: 8 q-blocks x full 8192 columns ----
            for qi, q0 in enumerate(QCOLS):
                p0 = (q0 + N2 // 2) % N2
                lhsT0 = h0b[:, q0:q0 + 128]
                lhsT1 = h1b[:, q0:q0 + 128]
                rsp = scrp.tile([128, NCH], F32, tag="rsp")
                for chx in range(NCH):
                    base = chx * CH
                    ps_t = psp.tile([128, CH], F32, tag="ps")
                    for s in range(NSUB):
                        c0 = base + s * SUB
                        ssl = slice(s * SUB, (s + 1) * SUB)
                        nc.tensor.matmul(ps_t[:, ssl], lhsT0,
                                         h0b[:, c0:c0 + SUB],
                                         start=True, stop=False)
                        if c0 <= q0 < c0 + SUB:
                            off = s * SUB + (q0 - c0)
                            nc.tensor.matmul(ps_t[:, off:off + 128],
                                             negidb_t[:], identb_t[:],
                                             start=False, stop=False,
                                             skip_group_check=True)
                        nc.tensor.matmul(ps_t[:, ssl], lhsT1,
                                         h1b[:, c0:c0 + SUB],
                                         start=False, stop=True)
                    es = esp.tile([128, CH], BF16, tag="es")
                    nc.scalar.activation(es[:], ps_t[:], AF.Exp,
                                         scale=float(1.0 / NCE_TEMP),
                                         accum_out=rsp[:, chx:chx + 1])
                    if base <= p0 < base + CH:
                        # NB: tensor_tensor_reduce faults this device -- use
                        # scalar_tensor_tensor to pick exp(pos) off the SBUF
                        # exp tile via the identity mask.
                        poff = p0 - base
                        pscr = scrp.tile([128, 128], F32, tag="posscr")
                        nc.vector.scalar_tensor_tensor(
                            out=pscr[:], in0=es[:, poff:poff + 128],
                            scalar=1.0, in1=ident_t[:],
                            op0=ALU.mult, op1=ALU.mult,
                            accum_out=rsq[:, 8 + qi:9 + qi])
                nc.vector.reduce_sum(rsq[:, qi:qi + 1], rsp[:], axis=AX.X)

            # ---- epilogue on packed stat vectors ----
            l4 = vecp.tile([128, 16], F32, tag="l4")
            nc.scalar.activation(l4[:], st4[:], AF.Ln)   # lse1|lseT|lsem|lsea
            l8 = vecp.tile([128, 16], F32, tag="l8")
            nc.scalar.activation(l8[:], rsq[:], AF.Ln)   # ln rs | ln posx

            ce = vecp.tile([128, NT], F32, tag="ce4")    # per-row CE (positive)
            nc.vector.tensor_sub(ce[:], l4[:, 0:4], gst[:, 0:4])
            adv = vecp.tile([128, NT], F32, tag="adv")
            nc.vector.tensor_sub(adv[:], l4[:, 12:16], gst[:, 4:8])
            nce = vecp.tile([128, QB], F32, tag="nce")
            nc.vector.tensor_sub(nce[:], l8[:, 0:8], l8[:, 8:16])

            # kl_row = PP/(T*SM) - lsem + lseT
            invSM = vecp.tile([128, NT], F32, tag="invSM")
            nc.vector.reciprocal(invSM[:], st4[:, 8:12])
            kl = vecp.tile([128, NT], F32, tag="kl")
            nc.vector.tensor_mul(kl[:], gst[:, 8:12], invSM[:])
            nc.vector.tensor_scalar_mul(kl[:], kl[:], float(1.0 / KL_TEMP))
            nc.vector.tensor_sub(kl[:], kl[:], l4[:, 8:12])
            nc.vector.tensor_add(kl[:], kl[:], l4[:, 4:8])

            # focal_row = (1-pt)^gamma * ce,  pt = exp(-ce)
            pt = vecp.tile([128, NT], F32, tag="pt")
            nc.scalar.activation(pt[:], ce[:], AF.Exp, scale=-1.0)
            c1 = vecp.tile([128, NT], F32, tag="c1")
            nc.vector.tensor_scalar(c1[:], pt[:], 0.5, None, op0=ALU.is_lt)
            c2 = vecp.tile([128, NT], F32, tag="c2")
            nc.vector.tensor_scalar(c2[:], pt[:], 0.2, None, op0=ALU.is_lt)
            gam = vecp.tile([128, NT], F32, tag="gam")
            nc.vector.tensor_add(gam[:], c1[:], c2[:])
            nc.vector.tensor_scalar(gam[:], gam[:], 2.0, 1.0,
                                    op0=ALU.mult, op1=ALU.add)
            u = vecp.tile([128, NT], F32, tag="u")       # 1 - pt
            nc.vector.tensor_scalar(u[:], pt[:], -1.0, 1.0,
                                    op0=ALU.mult, op1=ALU.add)
            lg = vecp.tile([128, NT], F32, tag="lg")
            nc.scalar.activation(lg[:], u[:], AF.Ln)
            w = vecp.tile([128, NT], F32, tag="w")
            nc.vector.tensor_mul(w[:], gam[:], lg[:])
            nc.scalar.activation(w[:], w[:], AF.Exp)     # (1-pt)^gamma
            foc = vecp.tile([128, NT], F32, tag="foc")
            nc.vector.tensor_mul(foc[:], w[:], ce[:])

            # ---- reduce to 5 partial sums, then across partitions via PE ----
            acc = vecp.tile([128, 8], F32, tag="acc")
            nc.vector.reduce_sum(acc[:, 0:1], kl[:], axis=AX.X)
            nc.vector.reduce_sum(acc[:, 1:2], ce[:], axis=AX.X)
            nc.vector.reduce_sum(acc[:, 2:3], adv[:], axis=AX.X)
            nc.vector.reduce_sum(acc[:, 3:4], foc[:], axis=AX.X)
            nc.vector.reduce_sum(acc[:, 4:5], nce[:], axis=AX.X)
            nc.vector.memset(acc[:, 5:8], 0.0)

            onesf = vecp.tile([128, 1], F32, tag="onesf")
            nc.vector.memset(onesf[:], 1.0)
            ps_f = psp.tile([8, 1], F32, tag="ps")
            nc.tensor.matmul(ps_f[:], acc[:], onesf[:],
                             start=True, stop=True)
            out_sb = vecp.tile([8, 1], F32, tag="out_sb")
            nc.scalar.copy(out_sb[:], ps_f[:])
            dma(out=res_d[:], in_=out_sb[:])

    nc.compile()
    return nc


_NC = None


def _get_nc():
    global _NC
    if _NC is None:
        _NC = _build_module()
    return _NC


def _prep_inputs(output, target, master_net_pred, feat_pooled,
                 feat_pooled_masked, output_adv, target_adv):
    import ml_dtypes
    bf16 = ml_dtypes.bfloat16
    o = np.asarray(output, dtype=np.float32)
    m = np.asarray(master_net_pred, dtype=np.float32)
    a = np.asarray(output_adv, dtype=np.float32)
    tg = np.asarray(target).astype(np.int64)
    ta = np.asarray(target_adv).astype(np.int64)
    f0 = np.asarray(feat_pooled, dtype=np.float32)
    f1 = np.asarray(feat_pooled_masked, dtype=np.float32)
    feats = np.concatenate([f0, f1], axis=0)  # [2B, D]

    def pack(x, sl):
        # [512, 1000] -> [128, 4*1000] bf16; col-block t holds row t*128+p
        return np.ascontiguousarray(
            x[sl].reshape(NT, 128, C).transpose(1, 0, 2).reshape(128, NT * C)
            .astype(bf16))

    in_maps = []
    for cc in range(NCORES):
        sl = slice(cc * RB, (cc + 1) * RB)
        roll = np.concatenate([np.arange(cc * RB, B), np.arange(0, cc * RB)])
        order = np.concatenate([roll, B + roll])
        ftc = feats[order].T.astype(bf16)  # [D, 2B], cols rolled
        in_maps.append({
            "o": pack(o, sl),
            "m": pack(m, sl),
            "a": pack(a, sl),
            "tg": np.ascontiguousarray(
                tg[sl].reshape(NT, 128).T.astype(np.float32)),
            "ta": np.ascontiguousarray(
                ta[sl].reshape(NT, 128).T.astype(np.float32)),
            "f0": np.ascontiguousarray(ftc[0:128]),
            "f1": np.ascontiguousarray(ftc[128:256]),
        })
    return in_maps


def _combine(results):
    r = np.zeros(8, dtype=np.float64)
    for rr in results:
        r += rr["res"].reshape(-1).astype(np.float64)
    kl_mean = r[0] / (B * C)
    ce_mean = r[1] / B
    adv_mean = r[2] / B
    foc_mean = r[3] / B
    nce_mean = r[4] / (2 * B)
    loss = (KL_INTERP * KL_TEMP * KL_TEMP) * kl_mean \
        + (1.0 - KL_INTERP) * ce_mean + nce_mean + foc_mean + adv_mean
    return np.asarray([loss], dtype=np.float32)


def kernel(**inputs):
    in_maps = _prep_inputs(**inputs)
    out = run_bass_kernel_spmd(_get_nc(), in_maps,
                               core_ids=list(range(NCORES)))
    return _combine(out.results)


if __name__ == "__main__":
    rng = np.random.default_rng(0)
    ins = {
        "output": rng.standard_normal((B, C), dtype=np.float32),
        "target": rng.integers(0, C, size=(B,)),
        "master_net_pred": rng.standard_normal((B, C), dtype=np.float32),
        "feat_pooled": rng.standard_normal((B, D), dtype=np.float32),
        "feat_pooled_masked": rng.standard_normal((B, D), dtype=np.float32),
        "output_adv": rng.standard_normal((B, C), dtype=np.float32),
        "target_adv": rng.integers(0, C, size=(B,)),
    }
    print(kernel(**ins))


# revision 11
# speedup vs baseline: 1.4885x; 1.0228x over previous
"""Trainium2 Bass kernel for the combined loss (KL + CE + InfoNCE + focal + adv CE).

Strategy (8 NeuronCores, data-parallel over the batch):
  - The three [4096, 1000] tensors (output / master_net_pred / output_adv) are
    sharded by rows (512 rows per core) and shipped as bf16 packed [128, 4*1000]
    (partition p, tile t holds row t*128+p).  Targets ship as f32 [128, 8].
  - InfoNCE: feats = concat(feat_pooled, feat_pooled_masked) -> [8192, 256].
    Every core receives the FULL feature matrix (bf16), transposed to
    [256, 8192] and with its columns rolled per core so its own 1024 query rows
    sit at columns {0..512, 4096..4608} -- one compiled program for all cores.
  - Engine plan (ScalarE's exp stream is the critical path; everything else is
    kept off it):
      * column norms: DVE squares (bf16 2x) -> bf16 ones-matmul partition sums
        -> Ln -> Exp(-0.5*ln) -> DVE normalize muls.
      * ACT instructions are grouped by table set ([Ln x4][Exp everything]
        [final packed Ln]) so only 3 ACT_TABLE_LOADs happen.
      * CE/KL/adv exps run batched as single [128, 4000] activations inside
        the initial DMA window; per-row sums via DVE reduces; gathers via
        iota==target masks with DVE accumulate.
      * focal (1-pt)^gamma uses DVE power/mask arithmetic (gamma in {1,3,5}),
        pt = exp(o[tgt]) * recip(sumexp) -- no extra Ln/Exp table switches.
      * InfoNCE: per 128-row q-block, 4 x [128, 2048] PSUM Gram chunks (bf16
        matmuls), fused exp(1/T * x) with row-sum accumulate; self-diagonal
        masked by accumulating -1e9*I via a tiny extra matmul; positives read
        off the exp tile with an identity mask.
      * DMA triggers are split across the Sync and GpSimd queues to halve the
        serial descriptor-issue time at kernel start.
  - Each core emits 5 partial sums; the host sums the 8x[8] outputs and applies
    the loss weights.
"""

import numpy as np

import concourse.bacc as bacc
import concourse.tile as tile
from concourse import mybir
from concourse.bass_utils import run_bass_kernel_spmd

F32 = mybir.dt.float32
BF16 = mybir.dt.bfloat16
AF = mybir.ActivationFunctionType
ALU = mybir.AluOpType
AX = mybir.AxisListType

NCORES = 8
B, C, D = 4096, 1000, 256
RB = B // NCORES          # 512 rows of the [B, C] tensors per core
NT = RB // 128            # 4 row-tiles per core
N2 = 2 * B                # 8192 infoNCE rows
CH = 2048                 # column chunk for the Gram stripe
NCH = N2 // CH            # 4 chunks
SUB = 512                 # matmul moving free dim (PSUM bank limit)
NSUB = CH // SUB
# q-block column starts after the per-core column roll (identical on all cores)
QCOLS = [0, 128, 256, 384, N2 // 2, N2 // 2 + 128, N2 // 2 + 256, N2 // 2 + 384]
QB = len(QCOLS)

KL_TEMP = 4.0
KL_INTERP = 0.5
NCE_TEMP = 0.07
NEG_BIG = -1.0e9


def _build_module():
    nc = bacc.Bacc("TRN2", target_bir_lowering=False, debug=False)

    o_d = nc.dram_tensor("o", [128, NT * C], BF16, kind="ExternalInput")
    m_d = nc.dram_tensor("m", [128, NT * C], BF16, kind="ExternalInput")
    a_d = nc.dram_tensor("a", [128, NT * C], BF16, kind="ExternalInput")
    tgta_d = nc.dram_tensor("tgta", [128, 2 * NT], F32, kind="ExternalInput")
    f0_d = nc.dram_tensor("f0", [128, N2], BF16, kind="ExternalInput")
    f1_d = nc.dram_tensor("f1", [128, N2], BF16, kind="ExternalInput")
    res_d = nc.dram_tensor("res", [8, 1], F32, kind="ExternalOutput")

    import ml_dtypes
    # packed constants: f32 [iota(1000) | ident(128)], bf16 [identb|negidb|ones]
    cstf_np = np.concatenate(
        [np.tile(np.arange(C, dtype=np.float32), (128, 1)),
         np.eye(128, dtype=np.float32)], axis=1)
    cstb_np = np.concatenate(
        [np.eye(128), NEG_BIG * np.eye(128), np.ones((128, 128))],
        axis=1).astype(ml_dtypes.bfloat16)
    cstf_d = nc.inline_tensor(cstf_np, "cstf")
    cstb_d = nc.inline_tensor(cstb_np, "cstb")

    with tile.TileContext(nc) as tc:
        with (
            tc.tile_pool(name="persist", bufs=1) as persist,
            tc.tile_pool(name="scr", bufs=4) as scrp,
            tc.tile_pool(name="cescr", bufs=3) as cescr,
            tc.tile_pool(name="rnlp", bufs=4) as rnlp,
            tc.tile_pool(name="vec", bufs=1) as vecp,
            tc.tile_pool(name="ps", bufs=2, space="PSUM") as psp,
        ):
            dma0 = nc.sync.dma_start
            dma1 = nc.gpsimd.dma_start

            # ---- tiles for inputs/consts ----
            cstf_t = persist.tile([128, C + 128], F32, tag="cstf")
            cstb_t = persist.tile([128, 384], BF16, tag="cstb")
            tgta_t = persist.tile([128, 2 * NT], F32, tag="tgta")
            raw_pool_cm = tc.tile_pool(name="raw", bufs=1)
            rawp = raw_pool_cm.__enter__()
            h0 = rawp.tile([128, N2], BF16, tag="h0")
            h1 = rawp.tile([128, N2], BF16, tag="h1")
            h0b = persist.tile([128, N2], BF16, tag="h0b")
            h1b = persist.tile([128, N2], BF16, tag="h1b")
            o_t = persist.tile([128, NT * C], BF16, tag="o")
            m_t = persist.tile([128, NT * C], BF16, tag="m")
            a_t = persist.tile([128, NT * C], BF16, tag="a")

            iota_t = cstf_t[:, 0:C]
            ident_t = cstf_t[:, C:C + 128]
            identb_t = cstb_t[:, 0:128]
            negidb_t = cstb_t[:, 128:256]
            onesb_t = cstb_t[:, 256:384]
            tg_t = tgta_t[:, 0:NT]
            ta_t = tgta_t[:, NT:2 * NT]

            # ---- DMA issue: split across two queues, compute-critical first
            def csl(j):
                return slice(j * CH, (j + 1) * CH)

            dma0(out=cstf_t[:], in_=cstf_d[:])
            dma1(out=cstb_t[:], in_=cstb_d[:])
            dma0(out=h0[:, csl(0)], in_=f0_d[:, csl(0)])
            dma1(out=h1[:, csl(0)], in_=f1_d[:, csl(0)])
            dma0(out=o_t[:], in_=o_d[:])
            dma1(out=tgta_t[:], in_=tgta_d[:])
            dma0(out=h0[:, csl(1)], in_=f0_d[:, csl(1)])
            dma1(out=h1[:, csl(1)], in_=f1_d[:, csl(1)])
            dma0(out=m_t[:], in_=m_d[:])
            dma1(out=h1[:, csl(2)], in_=f1_d[:, csl(2)])
            dma0(out=h0[:, csl(2)], in_=f0_d[:, csl(2)])
            dma1(out=h1[:, csl(3)], in_=f1_d[:, csl(3)])
            dma0(out=h0[:, csl(3)], in_=f0_d[:, csl(3)])
            dma1(out=a_t[:], in_=a_d[:])

            # ---- stat tiles ----
            stats = vecp.tile([128, 32], F32, tag="stats")  # S1|ST|SM|SA|rs|posx
            gst = vecp.tile([128, 12], F32, tag="gst")      # GO | GA | PP

            # ---- norm phase, Ln group first (one ACT table set) ----
            rls = []
            for j in range(NCH):
                sl = csl(j)
                s0 = scrp.tile([128, CH], BF16, tag="sq")
                nc.vector.tensor_mul(s0[:], h0[:, sl], h0[:, sl])
                s1 = scrp.tile([128, CH], BF16, tag="sq")
                nc.vector.tensor_mul(s1[:], h1[:, sl], h1[:, sl])
                ps_n = psp.tile([128, CH], F32, tag="ps")
                for s in range(NSUB):
                    ssl = slice(s * SUB, (s + 1) * SUB)
                    nc.tensor.matmul(ps_n[:, ssl], onesb_t, s0[:, ssl],
                                     start=True, stop=False)
                    nc.tensor.matmul(ps_n[:, ssl], onesb_t, s1[:, ssl],
                                     start=False, stop=True)
                rl = rnlp.tile([128, CH], BF16, tag="rnl")
                nc.scalar.activation(rl[:], ps_n[:], AF.Ln)
                rls.append(rl)

            # ---- Exp group: rn chunks -> normalize muls; cekl batched exps --
            for j in range(NCH):
                sl = csl(j)
                rnc = scrp.tile([128, CH], BF16, tag="rnc")
                nc.scalar.activation(rnc[:], rls[j][:], AF.Exp, scale=-0.5)
                nc.vector.tensor_mul(h0b[:, sl], h0[:, sl], rnc[:])
                nc.vector.tensor_mul(h1b[:, sl], h1[:, sl], rnc[:])

            e1 = cescr.tile([128, NT * C], BF16, tag="ce")
            nc.scalar.activation(e1[:], o_t[:], AF.Exp, scale=1.0)
            eT = cescr.tile([128, NT * C], BF16, tag="ce")
            nc.scalar.activation(eT[:], o_t[:], AF.Exp,
                                 scale=float(1.0 / KL_TEMP))
            em = persist.tile([128, NT * C], BF16, tag="em")
            nc.scalar.activation(em[:], m_t[:], AF.Exp,
                                 scale=float(1.0 / KL_TEMP))
            ea = cescr.tile([128, NT * C], BF16, tag="ce")
            nc.scalar.activation(ea[:], a_t[:], AF.Exp, scale=1.0)

            # raw features are dead once normalized -- release their SBUF
            # before the NCE exp-tile pool opens.
            raw_pool_cm.__exit__(None, None, None)
            import contextlib
            esp_cm = tc.tile_pool(name="esp", bufs=3)
            esp = esp_cm.__enter__()

            # ---- cekl DVE chain (fills DVE while ACT grinds the Gram exps) --
            for t in range(NT):
                ccl = slice(t * C, (t + 1) * C)
                nc.vector.reduce_sum(stats[:, t:t + 1], e1[:, ccl], axis=AX.X)
                nc.vector.reduce_sum(stats[:, 4 + t:5 + t], eT[:, ccl],
                                     axis=AX.X)
                nc.vector.reduce_sum(stats[:, 8 + t:9 + t], em[:, ccl],
                                     axis=AX.X)
            d_t = cescr.tile([128, NT * C], BF16, tag="d")
            nc.vector.tensor_sub(d_t[:], m_t[:], o_t[:])
            for t in range(NT):
                ccl = slice(t * C, (t + 1) * C)
                pr = scrp.tile([128, C], BF16, tag="pr")
                nc.vector.scalar_tensor_tensor(
                    out=pr[:], in0=d_t[:, ccl], scalar=1.0, in1=em[:, ccl],
                    op0=ALU.mult, op1=ALU.mult, accum_out=gst[:, 8 + t:9 + t])
                g1 = scrp.tile([128, C], BF16, tag="pr")
                nc.vector.scalar_tensor_tensor(
                    out=g1[:], in0=iota_t, scalar=tg_t[:, t:t + 1],
                    in1=o_t[:, ccl], op0=ALU.is_equal, op1=ALU.mult,
                    accum_out=gst[:, t:t + 1])
                g2 = scrp.tile([128, C], BF16, tag="pr")
                nc.vector.scalar_tensor_tensor(
                    out=g2[:], in0=iota_t, scalar=ta_t[:, t:t + 1],
                    in1=a_t[:, ccl], op0=ALU.is_equal, op1=ALU.mult,
                    accum_out=gst[:, 4 + t:5 + t])
                nc.vector.reduce_sum(stats[:, 12 + t:13 + t], ea[:, ccl],
                                     axis=AX.X)

            # pt numerator exp(o[tgt]) -- stays inside the Exp table group
            ptn = vecp.tile([128, NT], F32, tag="ptn")
            nc.scalar.activation(ptn[:], gst[:, 0:4], AF.Exp)

            # ---- InfoNCE stripe: 8 q-blocks x full 8192 columns ----
            for qi, q0 in enumerate(QCOLS):
                p0 = (q0 + N2 // 2) % N2
                lhsT0 = h0b[:, q0:q0 + 128]
                lhsT1 = h1b[:, q0:q0 + 128]
                rsp = scrp.tile([128, NCH], F32, tag="rsp")
                for chx in range(NCH):
                    base = chx * CH
                    ps_t = psp.tile([128, CH], F32, tag="ps")
                    for s in range(NSUB):
                        c0 = base + s * SUB
                        ssl = slice(s * SUB, (s + 1) * SUB)
                        nc.tensor.matmul(ps_t[:, ssl], lhsT0,
                                         h0b[:, c0:c0 + SUB],
                                         start=True, stop=False)
                        if c0 <= q0 < c0 + SUB:
                            off = s * SUB + (q0 - c0)
                            nc.tensor.matmul(ps_t[:, off:off + 128],
                                             negidb_t, identb_t,
                                             start=False, stop=False,
                                             skip_group_check=True)
                        nc.tensor.matmul(ps_t[:, ssl], lhsT1,
                                         h1b[:, c0:c0 + SUB],
                                         start=False, stop=True)
                    es = esp.tile([128, CH], BF16, tag="es")
                    nc.scalar.activation(es[:], ps_t[:], AF.Exp,
                                         scale=float(1.0 / NCE_TEMP),
                                         accum_out=rsp[:, chx:chx + 1])
                    if base <= p0 < base + CH:
                        # NB: tensor_tensor_reduce faults this device -- use
                        # scalar_tensor_tensor to pick exp(pos) off the SBUF
                        # exp tile via the identity mask.
                        poff = p0 - base
                        pscr = scrp.tile([128, 128], F32, tag="posscr")
                        nc.vector.scalar_tensor_tensor(
                            out=pscr[:], in0=es[:, poff:poff + 128],
                            scalar=1.0, in1=ident_t,
                            op0=ALU.mult, op1=ALU.mult,
                            accum_out=stats[:, 24 + qi:25 + qi])
                nc.vector.reduce_sum(stats[:, 16 + qi:17 + qi], rsp[:],
                                     axis=AX.X)

            # ---- epilogue: one packed Ln, rest on DVE ----
            lall = vecp.tile([128, 32], F32, tag="lall")
            nc.scalar.activation(lall[:], stats[:], AF.Ln)

            ce = vecp.tile([128, NT], F32, tag="ce4")    # per-row CE (positive)
            nc.vector.tensor_sub(ce[:], lall[:, 0:4], gst[:, 0:4])
            adv = vecp.tile([128, NT], F32, tag="adv")
            nc.vector.tensor_sub(adv[:], lall[:, 12:16], gst[:, 4:8])
            nce = vecp.tile([128, QB], F32, tag="nce")
            nc.vector.tensor_sub(nce[:], lall[:, 16:24], lall[:, 24:32])

            # kl_row = PP/(T*SM) - lsem + lseT
            invSM = vecp.tile([128, NT], F32, tag="invSM")
            nc.vector.reciprocal(invSM[:], stats[:, 8:12])
            kl = vecp.tile([128, NT], F32, tag="kl")
            nc.vector.tensor_mul(kl[:], gst[:, 8:12], invSM[:])
            nc.vector.tensor_scalar_mul(kl[:], kl[:], float(1.0 / KL_TEMP))
            nc.vector.tensor_sub(kl[:], kl[:], lall[:, 8:12])
            nc.vector.tensor_add(kl[:], kl[:], lall[:, 4:8])

            # focal_row = (1-pt)^gamma * ce with gamma in {1,3,5}:
            # w = u + c1*(u^3-u) + c2*(u^5-u^3), all on DVE (no table switch)
            pt = vecp.tile([128, NT], F32, tag="pt")
            invS1 = vecp.tile([128, NT], F32, tag="invS1")
            nc.vector.reciprocal(invS1[:], stats[:, 0:4])
            nc.vector.tensor_mul(pt[:], ptn[:], invS1[:])
            c1 = vecp.tile([128, NT], F32, tag="c1")
            nc.vector.tensor_scalar(c1[:], pt[:], 0.5, None, op0=ALU.is_lt)
            c2 = vecp.tile([128, NT], F32, tag="c2")
            nc.vector.tensor_scalar(c2[:], pt[:], 0.2, None, op0=ALU.is_lt)
            u = vecp.tile([128, NT], F32, tag="u")       # 1 - pt
            nc.vector.tensor_scalar(u[:], pt[:], -1.0, 1.0,
                                    op0=ALU.mult, op1=ALU.add)
            u2 = vecp.tile([128, NT], F32, tag="u2")
            nc.vector.tensor_mul(u2[:], u[:], u[:])
            u3 = vecp.tile([128, NT], F32, tag="u3")
            nc.vector.tensor_mul(u3[:], u2[:], u[:])
            u5 = vecp.tile([128, NT], F32, tag="u5")
            nc.vector.tensor_mul(u5[:], u2[:], u3[:])
            ta_3 = vecp.tile([128, NT], F32, tag="ta3")
            nc.vector.tensor_sub(ta_3[:], u3[:], u[:])
            tb_5 = vecp.tile([128, NT], F32, tag="tb5")
            nc.vector.tensor_sub(tb_5[:], u5[:], u3[:])
            w = vecp.tile([128, NT], F32, tag="w")
            nc.vector.tensor_mul(w[:], c1[:], ta_3[:])
            nc.vector.tensor_add(w[:], w[:], u[:])
            wb = vecp.tile([128, NT], F32, tag="wb")
            nc.vector.tensor_mul(wb[:], c2[:], tb_5[:])
            nc.vector.tensor_add(w[:], w[:], wb[:])
            foc = vecp.tile([128, NT], F32, tag="foc")
            nc.vector.tensor_mul(foc[:], w[:], ce[:])

            # ---- reduce to 5 partial sums, then across partitions via PE ----
            acc = vecp.tile([128, 8], F32, tag="acc")
            nc.vector.reduce_sum(acc[:, 0:1], kl[:], axis=AX.X)
            nc.vector.reduce_sum(acc[:, 1:2], ce[:], axis=AX.X)
            nc.vector.reduce_sum(acc[:, 2:3], adv[:], axis=AX.X)
            nc.vector.reduce_sum(acc[:, 3:4], foc[:], axis=AX.X)
            nc.vector.reduce_sum(acc[:, 4:5], nce[:], axis=AX.X)
            nc.vector.memset(acc[:, 5:8], 0.0)

            onesf = vecp.tile([128, 1], F32, tag="onesf")
            nc.vector.memset(onesf[:], 1.0)
            ps_f = psp.tile([8, 1], F32, tag="ps")
            nc.tensor.matmul(ps_f[:], acc[:], onesf[:],
                             start=True, stop=True)
            out_sb = vecp.tile([8, 1], F32, tag="out_sb")
            nc.scalar.copy(out_sb[:], ps_f[:])
            dma0(out=res_d[:], in_=out_sb[:])

            esp_cm.__exit__(None, None, None)

    nc.compile()
    return nc


_NC = None


def _get_nc():
    global _NC
    if _NC is None:
        _NC = _build_module()
    return _NC


def _prep_inputs(output, target, master_net_pred, feat_pooled,
                 feat_pooled_masked, output_adv, target_adv):
    import ml_dtypes
    bf16 = ml_dtypes.bfloat16
    o = np.asarray(output, dtype=np.float32)
    m = np.asarray(master_net_pred, dtype=np.float32)
    a = np.asarray(output_adv, dtype=np.float32)
    tg = np.asarray(target).astype(np.int64)
    ta = np.asarray(target_adv).astype(np.int64)
    f0 = np.asarray(feat_pooled, dtype=np.float32)
    f1 = np.asarray(feat_pooled_masked, dtype=np.float32)
    feats = np.concatenate([f0, f1], axis=0)  # [2B, D]

    def pack(x, sl):
        # [512, 1000] -> [128, 4*1000] bf16; col-block t holds row t*128+p
        return np.ascontiguousarray(
            x[sl].reshape(NT, 128, C).transpose(1, 0, 2).reshape(128, NT * C)
            .astype(bf16))

    in_maps = []
    for cc in range(NCORES):
        sl = slice(cc * RB, (cc + 1) * RB)
        roll = np.concatenate([np.arange(cc * RB, B), np.arange(0, cc * RB)])
        order = np.concatenate([roll, B + roll])
        ftc = feats[order].T.astype(bf16)  # [D, 2B], cols rolled
        tgta = np.concatenate([tg[sl].reshape(NT, 128).T,
                               ta[sl].reshape(NT, 128).T],
                              axis=1).astype(np.float32)
        in_maps.append({
            "o": pack(o, sl),
            "m": pack(m, sl),
            "a": pack(a, sl),
            "tgta": np.ascontiguousarray(tgta),
            "f0": np.ascontiguousarray(ftc[0:128]),
            "f1": np.ascontiguousarray(ftc[128:256]),
        })
    return in_maps


def _combine(results):
    r = np.zeros(8, dtype=np.float64)
    for rr in results:
        r += rr["res"].reshape(-1).astype(np.float64)
    kl_mean = r[0] / (B * C)
    ce_mean = r[1] / B
    adv_mean = r[2] / B
    foc_mean = r[3] / B
    nce_mean = r[4] / (2 * B)
    loss = (KL_INTERP * KL_TEMP * KL_TEMP) * kl_mean \
        + (1.0 - KL_INTERP) * ce_mean + nce_mean + foc_mean + adv_mean
    return np.asarray([loss], dtype=np.float32)


def kernel(**inputs):
    in_maps = _prep_inputs(**inputs)
    out = run_bass_kernel_spmd(_get_nc(), in_maps,
                               core_ids=list(range(NCORES)))
    return _combine(out.results)


if __name__ == "__main__":
    rng = np.random.default_rng(0)
    ins = {
        "output": rng.standard_normal((B, C), dtype=np.float32),
        "target": rng.integers(0, C, size=(B,)),
        "master_net_pred": rng.standard_normal((B, C), dtype=np.float32),
        "feat_pooled": rng.standard_normal((B, D), dtype=np.float32),
        "feat_pooled_masked": rng.standard_normal((B, D), dtype=np.float32),
        "output_adv": rng.standard_normal((B, C), dtype=np.float32),
        "target_adv": rng.integers(0, C, size=(B,)),
    }
    print(kernel(**ins))


# revision 15
# speedup vs baseline: 1.5002x; 1.0078x over previous
"""Trainium2 Bass kernel for the combined loss (KL + CE + InfoNCE + focal + adv CE).

Strategy (8 NeuronCores, data-parallel over the batch):
  - The three [4096, 1000] tensors (output / master_net_pred / output_adv) are
    sharded by rows (512 rows per core) and shipped as bf16 packed [128, 4*1000]
    (partition p, tile t holds row t*128+p).  Targets ship as f32 [128, 8].
  - InfoNCE: feats = concat(feat_pooled, feat_pooled_masked) -> [8192, 256].
    Every core receives the FULL feature matrix (bf16), transposed to
    [256, 8192] and with its columns rolled per core so its own 1024 query rows
    sit at columns {0..512, 4096..4608} -- one compiled program for all cores.
  - Engine plan (ScalarE's exp stream is the critical path; everything else is
    kept off it):
      * column norms: DVE squares (bf16 2x) -> bf16 ones-matmul partition sums
        -> Ln -> Exp(-0.5*ln) -> DVE normalize muls.
      * ACT instructions are grouped by table set ([Ln x4][Exp everything]
        [final packed Ln]) so only 3 ACT_TABLE_LOADs happen.
      * CE/KL/adv exps run batched as single [128, 4000] activations inside
        the initial DMA window; per-row sums via DVE reduces; gathers via
        iota==target masks with DVE accumulate.
      * focal (1-pt)^gamma uses DVE power/mask arithmetic (gamma in {1,3,5}),
        pt = exp(o[tgt]) * recip(sumexp) -- no extra Ln/Exp table switches.
      * InfoNCE: per 128-row q-block, 4 x [128, 2048] PSUM Gram chunks (bf16
        matmuls), fused exp(1/T * x) with row-sum accumulate; self-diagonal
        masked by accumulating -1e9*I via a tiny extra matmul; positives read
        off the exp tile with an identity mask.
      * DMA triggers are split across the Sync and GpSimd queues to halve the
        serial descriptor-issue time at kernel start.
  - Each core emits 5 partial sums; the host sums the 8x[8] outputs and applies
    the loss weights.
"""

import numpy as np

import concourse.bacc as bacc
import concourse.tile as tile
from concourse import mybir
from concourse.bass_utils import run_bass_kernel_spmd

F32 = mybir.dt.float32
BF16 = mybir.dt.bfloat16
AF = mybir.ActivationFunctionType
ALU = mybir.AluOpType
AX = mybir.AxisListType

NCORES = 8
B, C, D = 4096, 1000, 256
RB = B // NCORES          # 512 rows of the [B, C] tensors per core
NT = RB // 128            # 4 row-tiles per core
N2 = 2 * B                # 8192 infoNCE rows
CH = 2048                 # column chunk for the Gram stripe
NCH = N2 // CH            # 4 chunks
SUB = 512                 # matmul moving free dim (PSUM bank limit)
NSUB = CH // SUB
# q-block column starts after the per-core column roll (identical on all cores)
QCOLS = [0, 128, 256, 384, N2 // 2, N2 // 2 + 128, N2 // 2 + 256, N2 // 2 + 384]
QB = len(QCOLS)

KL_TEMP = 4.0
KL_INTERP = 0.5
NCE_TEMP = 0.07
NEG_BIG = -1.0e9


def _build_module():
    nc = bacc.Bacc("TRN2", target_bir_lowering=False, debug=False)

    o_d = nc.dram_tensor("o", [128, NT * C], BF16, kind="ExternalInput")
    m_d = nc.dram_tensor("m", [128, NT * C], BF16, kind="ExternalInput")
    a_d = nc.dram_tensor("a", [128, NT * C], BF16, kind="ExternalInput")
    tgta_d = nc.dram_tensor("tgta", [128, 2 * NT], F32, kind="ExternalInput")
    f0_d = nc.dram_tensor("f0", [128, N2], BF16, kind="ExternalInput")
    f1_d = nc.dram_tensor("f1", [128, N2], BF16, kind="ExternalInput")
    res_d = nc.dram_tensor("res", [8, 1], F32, kind="ExternalOutput")

    import ml_dtypes
    # packed constants: f32 [iota(1000) | ident(128)], bf16 [identb|negidb|ones]
    cstf_np = np.concatenate(
        [np.tile(np.arange(C, dtype=np.float32), (128, 1)),
         np.eye(128, dtype=np.float32)], axis=1)
    cstb_np = np.concatenate(
        [np.eye(128), NEG_BIG * np.eye(128), np.ones((128, 128))],
        axis=1).astype(ml_dtypes.bfloat16)
    cstf_d = nc.inline_tensor(cstf_np, "cstf")
    cstb_d = nc.inline_tensor(cstb_np, "cstb")

    with tile.TileContext(nc) as tc:
        with (
            tc.tile_pool(name="persist", bufs=1) as persist,
            tc.tile_pool(name="scr", bufs=4) as scrp,
            tc.tile_pool(name="cescr", bufs=3) as cescr,
            tc.tile_pool(name="rnlp", bufs=4) as rnlp,
            tc.tile_pool(name="vec", bufs=1) as vecp,
            tc.tile_pool(name="ps", bufs=2, space="PSUM") as psp,
        ):
            dma0 = nc.sync.dma_start
            dma1 = nc.sync.dma_start

            # ---- tiles for inputs/consts ----
            cstf_t = persist.tile([128, C + 128], F32, tag="cstf")
            cstb_t = persist.tile([128, 384], BF16, tag="cstb")
            tgta_t = persist.tile([128, 2 * NT], F32, tag="tgta")
            raw_pool_cm = tc.tile_pool(name="raw", bufs=1)
            rawp = raw_pool_cm.__enter__()
            h0 = rawp.tile([128, N2], BF16, tag="h0")
            h1 = rawp.tile([128, N2], BF16, tag="h1")
            h0b = persist.tile([128, N2], BF16, tag="h0b")
            h1b = persist.tile([128, N2], BF16, tag="h1b")
            o_t = persist.tile([128, NT * C], BF16, tag="o")
            m_t = persist.tile([128, NT * C], BF16, tag="m")
            a_t = persist.tile([128, NT * C], BF16, tag="a")

            iota_t = cstf_t[:, 0:C]
            ident_t = cstf_t[:, C:C + 128]
            identb_t = cstb_t[:, 0:128]
            negidb_t = cstb_t[:, 128:256]
            onesb_t = cstb_t[:, 256:384]
            tg_t = tgta_t[:, 0:NT]
            ta_t = tgta_t[:, NT:2 * NT]

            # ---- DMA issue: split across two queues, compute-critical first
            def csl(j):
                return slice(j * CH, (j + 1) * CH)

            dma0(out=h0[:, csl(0)], in_=f0_d[:, csl(0)])
            dma1(out=h1[:, csl(0)], in_=f1_d[:, csl(0)])
            dma0(out=cstb_t[:], in_=cstb_d[:])
            dma1(out=tgta_t[:], in_=tgta_d[:])
            dma0(out=o_t[:], in_=o_d[:])
            dma1(out=h1[:, csl(1)], in_=f1_d[:, csl(1)])
            dma0(out=h0[:, csl(1)], in_=f0_d[:, csl(1)])
            dma1(out=h1[:, csl(2)], in_=f1_d[:, csl(2)])
            dma0(out=cstf_t[:], in_=cstf_d[:])
            dma0(out=m_t[:], in_=m_d[:])
            dma1(out=h1[:, csl(3)], in_=f1_d[:, csl(3)])
            dma0(out=h0[:, csl(2)], in_=f0_d[:, csl(2)])
            dma0(out=h0[:, csl(3)], in_=f0_d[:, csl(3)])
            dma1(out=a_t[:], in_=a_d[:])

            # ---- stat tiles ----
            stats = vecp.tile([128, 32], F32, tag="stats")  # S1|ST|SM|SA|rs|posx
            gst = vecp.tile([128, 12], F32, tag="gst")      # GO | GA | PP

            # ---- norm phase, Ln group first (one ACT table set) ----
            rls = []
            for j in range(NCH):
                sl = csl(j)
                s0 = scrp.tile([128, CH], BF16, tag="sq")
                nc.vector.tensor_mul(s0[:], h0[:, sl], h0[:, sl])
                s1 = scrp.tile([128, CH], BF16, tag="sq")
                nc.vector.tensor_mul(s1[:], h1[:, sl], h1[:, sl])
                ps_n = psp.tile([128, CH], F32, tag="ps")
                for s in range(NSUB):
                    ssl = slice(s * SUB, (s + 1) * SUB)
                    nc.tensor.matmul(ps_n[:, ssl], onesb_t, s0[:, ssl],
                                     start=True, stop=False)
                    nc.tensor.matmul(ps_n[:, ssl], onesb_t, s1[:, ssl],
                                     start=False, stop=True)
                rl = rnlp.tile([128, CH], BF16, tag="rnl")
                nc.scalar.activation(rl[:], ps_n[:], AF.Ln)
                rls.append(rl)

            # ---- Exp group: rn chunks -> normalize muls; cekl batched exps --
            for j in range(NCH):
                sl = csl(j)
                rnc = scrp.tile([128, CH], BF16, tag="rnc")
                nc.scalar.activation(rnc[:], rls[j][:], AF.Exp, scale=-0.5)
                nc.vector.tensor_mul(h0b[:, sl], h0[:, sl], rnc[:])
                nc.vector.tensor_mul(h1b[:, sl], h1[:, sl], rnc[:])

            e1 = cescr.tile([128, NT * C], BF16, tag="ce")
            nc.scalar.activation(e1[:], o_t[:], AF.Exp, scale=1.0)
            eT = cescr.tile([128, NT * C], BF16, tag="ce")
            nc.scalar.activation(eT[:], o_t[:], AF.Exp,
                                 scale=float(1.0 / KL_TEMP))
            em = persist.tile([128, NT * C], BF16, tag="em")
            nc.scalar.activation(em[:], m_t[:], AF.Exp,
                                 scale=float(1.0 / KL_TEMP))
            ea = cescr.tile([128, NT * C], BF16, tag="ce")
            nc.scalar.activation(ea[:], a_t[:], AF.Exp, scale=1.0)

            # raw features are dead once normalized -- release their SBUF
            # before the NCE exp-tile pool opens.
            raw_pool_cm.__exit__(None, None, None)
            import contextlib
            esp_cm = tc.tile_pool(name="esp", bufs=6)
            esp = esp_cm.__enter__()

            # ---- cekl DVE chain (fills DVE while ACT grinds the Gram exps) --
            for t in range(NT):
                ccl = slice(t * C, (t + 1) * C)
                nc.vector.reduce_sum(stats[:, t:t + 1], e1[:, ccl], axis=AX.X)
                nc.vector.reduce_sum(stats[:, 4 + t:5 + t], eT[:, ccl],
                                     axis=AX.X)
                nc.vector.reduce_sum(stats[:, 8 + t:9 + t], em[:, ccl],
                                     axis=AX.X)
            d_t = cescr.tile([128, NT * C], BF16, tag="d")
            nc.vector.tensor_sub(d_t[:], m_t[:], o_t[:])
            for t in range(NT):
                ccl = slice(t * C, (t + 1) * C)
                pr = scrp.tile([128, C], BF16, tag="pr")
                nc.vector.scalar_tensor_tensor(
                    out=pr[:], in0=d_t[:, ccl], scalar=1.0, in1=em[:, ccl],
                    op0=ALU.mult, op1=ALU.mult, accum_out=gst[:, 8 + t:9 + t])
                g1 = scrp.tile([128, C], BF16, tag="pr")
                nc.vector.scalar_tensor_tensor(
                    out=g1[:], in0=iota_t, scalar=tg_t[:, t:t + 1],
                    in1=o_t[:, ccl], op0=ALU.is_equal, op1=ALU.mult,
                    accum_out=gst[:, t:t + 1])
                g2 = scrp.tile([128, C], BF16, tag="pr")
                nc.vector.scalar_tensor_tensor(
                    out=g2[:], in0=iota_t, scalar=ta_t[:, t:t + 1],
                    in1=a_t[:, ccl], op0=ALU.is_equal, op1=ALU.mult,
                    accum_out=gst[:, 4 + t:5 + t])
                nc.vector.reduce_sum(stats[:, 12 + t:13 + t], ea[:, ccl],
                                     axis=AX.X)

            # pt numerator exp(o[tgt]) -- stays inside the Exp table group
            ptn = vecp.tile([128, NT], F32, tag="ptn")
            nc.scalar.activation(ptn[:], gst[:, 0:4], AF.Exp)

            # ---- InfoNCE stripe: 8 q-blocks x full 8192 columns ----
            for qi, q0 in enumerate(QCOLS):
                p0 = (q0 + N2 // 2) % N2
                lhsT0 = h0b[:, q0:q0 + 128]
                lhsT1 = h1b[:, q0:q0 + 128]
                rsp = scrp.tile([128, NCH], F32, tag="rsp")
                for chx in range(NCH):
                    base = chx * CH
                    ps_t = psp.tile([128, CH], F32, tag="ps")
                    for s in range(NSUB):
                        c0 = base + s * SUB
                        ssl = slice(s * SUB, (s + 1) * SUB)
                        nc.tensor.matmul(ps_t[:, ssl], lhsT0,
                                         h0b[:, c0:c0 + SUB],
                                         start=True, stop=False)
                        if c0 <= q0 < c0 + SUB:
                            off = s * SUB + (q0 - c0)
                            nc.tensor.matmul(ps_t[:, off:off + 128],
                                             negidb_t, identb_t,
                                             start=False, stop=False,
                                             skip_group_check=True)
                        nc.tensor.matmul(ps_t[:, ssl], lhsT1,
                                         h1b[:, c0:c0 + SUB],
                                         start=False, stop=True)
                    es = esp.tile([128, CH], BF16, tag="es")
                    nc.scalar.activation(es[:], ps_t[:], AF.Exp,
                                         scale=float(1.0 / NCE_TEMP),
                                         accum_out=rsp[:, chx:chx + 1])
                    if base <= p0 < base + CH:
                        # NB: tensor_tensor_reduce faults this device -- use
                        # scalar_tensor_tensor to pick exp(pos) off the SBUF
                        # exp tile via the identity mask.
                        poff = p0 - base
                        pscr = scrp.tile([128, 128], F32, tag="posscr")
                        nc.vector.scalar_tensor_tensor(
                            out=pscr[:], in0=es[:, poff:poff + 128],
                            scalar=1.0, in1=ident_t,
                            op0=ALU.mult, op1=ALU.mult,
                            accum_out=stats[:, 24 + qi:25 + qi])
                nc.vector.reduce_sum(stats[:, 16 + qi:17 + qi], rsp[:],
                                     axis=AX.X)

            # ---- epilogue: one packed Ln, rest on DVE ----
            lall = vecp.tile([128, 32], F32, tag="lall")
            nc.scalar.activation(lall[:], stats[:], AF.Ln)

            ce = vecp.tile([128, NT], F32, tag="ce4")    # per-row CE (positive)
            nc.vector.tensor_sub(ce[:], lall[:, 0:4], gst[:, 0:4])
            adv = vecp.tile([128, NT], F32, tag="adv")
            nc.vector.tensor_sub(adv[:], lall[:, 12:16], gst[:, 4:8])
            nce = vecp.tile([128, QB], F32, tag="nce")
            nc.vector.tensor_sub(nce[:], lall[:, 16:24], lall[:, 24:32])

            # kl_row = PP/(T*SM) - lsem + lseT
            invSM = vecp.tile([128, NT], F32, tag="invSM")
            nc.vector.reciprocal(invSM[:], stats[:, 8:12])
            kl = vecp.tile([128, NT], F32, tag="kl")
            nc.vector.tensor_mul(kl[:], gst[:, 8:12], invSM[:])
            nc.vector.tensor_scalar_mul(kl[:], kl[:], float(1.0 / KL_TEMP))
            nc.vector.tensor_sub(kl[:], kl[:], lall[:, 8:12])
            nc.vector.tensor_add(kl[:], kl[:], lall[:, 4:8])

            # focal_row = (1-pt)^gamma * ce with gamma in {1,3,5}:
            # w = u + c1*(u^3-u) + c2*(u^5-u^3), all on DVE (no table switch)
            pt = vecp.tile([128, NT], F32, tag="pt")
            invS1 = vecp.tile([128, NT], F32, tag="invS1")
            nc.vector.reciprocal(invS1[:], stats[:, 0:4])
            nc.vector.tensor_mul(pt[:], ptn[:], invS1[:])
            c1 = vecp.tile([128, NT], F32, tag="c1")
            nc.vector.tensor_scalar(c1[:], pt[:], 0.5, None, op0=ALU.is_lt)
            c2 = vecp.tile([128, NT], F32, tag="c2")
            nc.vector.tensor_scalar(c2[:], pt[:], 0.2, None, op0=ALU.is_lt)
            u = vecp.tile([128, NT], F32, tag="u")       # 1 - pt
            nc.vector.tensor_scalar(u[:], pt[:], -1.0, 1.0,
                                    op0=ALU.mult, op1=ALU.add)
            u2 = vecp.tile([128, NT], F32, tag="u2")
            nc.vector.tensor_mul(u2[:], u[:], u[:])
            u3 = vecp.tile([128, NT], F32, tag="u3")
            nc.vector.tensor_mul(u3[:], u2[:], u[:])
            u5 = vecp.tile([128, NT], F32, tag="u5")
            nc.vector.tensor_mul(u5[:], u2[:], u3[:])
            ta_3 = vecp.tile([128, NT], F32, tag="ta3")
            nc.vector.tensor_sub(ta_3[:], u3[:], u[:])
            tb_5 = vecp.tile([128, NT], F32, tag="tb5")
            nc.vector.tensor_sub(tb_5[:], u5[:], u3[:])
            w = vecp.tile([128, NT], F32, tag="w")
            nc.vector.tensor_mul(w[:], c1[:], ta_3[:])
            nc.vector.tensor_add(w[:], w[:], u[:])
            wb = vecp.tile([128, NT], F32, tag="wb")
            nc.vector.tensor_mul(wb[:], c2[:], tb_5[:])
            nc.vector.tensor_add(w[:], w[:], wb[:])
            foc = vecp.tile([128, NT], F32, tag="foc")
            nc.vector.tensor_mul(foc[:], w[:], ce[:])

            # ---- reduce to 5 partial sums, then across partitions via PE ----
            acc = vecp.tile([128, 8], F32, tag="acc")
            nc.vector.reduce_sum(acc[:, 0:1], kl[:], axis=AX.X)
            nc.vector.reduce_sum(acc[:, 1:2], ce[:], axis=AX.X)
            nc.vector.reduce_sum(acc[:, 2:3], adv[:], axis=AX.X)
            nc.vector.reduce_sum(acc[:, 3:4], foc[:], axis=AX.X)
            nc.vector.reduce_sum(acc[:, 4:5], nce[:], axis=AX.X)
            nc.vector.memset(acc[:, 5:8], 0.0)

            onesf = vecp.tile([128, 1], F32, tag="onesf")
            nc.vector.memset(onesf[:], 1.0)
            ps_f = psp.tile([8, 1], F32, tag="ps")
            nc.tensor.matmul(ps_f[:], acc[:], onesf[:],
                             start=True, stop=True)
            out_sb = vecp.tile([8, 1], F32, tag="out_sb")
            nc.scalar.copy(out_sb[:], ps_f[:])
            dma0(out=res_d[:], in_=out_sb[:])

            esp_cm.__exit__(None, None, None)

    nc.compile()
    return nc


_NC = None


def _get_nc():
    global _NC
    if _NC is None:
        _NC = _build_module()
    return _NC


def _prep_inputs(output, target, master_net_pred, feat_pooled,
                 feat_pooled_masked, output_adv, target_adv):
    import ml_dtypes
    bf16 = ml_dtypes.bfloat16
    o = np.asarray(output, dtype=np.float32)
    m = np.asarray(master_net_pred, dtype=np.float32)
    a = np.asarray(output_adv, dtype=np.float32)
    tg = np.asarray(target).astype(np.int64)
    ta = np.asarray(target_adv).astype(np.int64)
    f0 = np.asarray(feat_pooled, dtype=np.float32)
    f1 = np.asarray(feat_pooled_masked, dtype=np.float32)
    feats = np.concatenate([f0, f1], axis=0)  # [2B, D]

    def pack(x, sl):
        # [512, 1000] -> [128, 4*1000] bf16; col-block t holds row t*128+p
        return np.ascontiguousarray(
            x[sl].reshape(NT, 128, C).transpose(1, 0, 2).reshape(128, NT * C)
            .astype(bf16))

    in_maps = []
    for cc in range(NCORES):
        sl = slice(cc * RB, (cc + 1) * RB)
        roll = np.concatenate([np.arange(cc * RB, B), np.arange(0, cc * RB)])
        order = np.concatenate([roll, B + roll])
        ftc = feats[order].T.astype(bf16)  # [D, 2B], cols rolled
        tgta = np.concatenate([tg[sl].reshape(NT, 128).T,
                               ta[sl].reshape(NT, 128).T],
                              axis=1).astype(np.float32)
        in_maps.append({
            "o": pack(o, sl),
            "m": pack(m, sl),
            "a": pack(a, sl),
            "tgta": np.ascontiguousarray(tgta),
            "f0": np.ascontiguousarray(ftc[0:128]),
            "f1": np.ascontiguousarray(ftc[128:256]),
        })
    return in_maps


def _combine(results):
    r = np.zeros(8, dtype=np.float64)
    for rr in results:
        r += rr["res"].reshape(-1).astype(np.float64)
    kl_mean = r[0] / (B * C)
    ce_mean = r[1] / B
    adv_mean = r[2] / B
    foc_mean = r[3] / B
    nce_mean = r[4] / (2 * B)
    loss = (KL_INTERP * KL_TEMP * KL_TEMP) * kl_mean \
        + (1.0 - KL_INTERP) * ce_mean + nce_mean + foc_mean + adv_mean
    return np.asarray([loss], dtype=np.float32)


def kernel(**inputs):
    in_maps = _prep_inputs(**inputs)
    out = run_bass_kernel_spmd(_get_nc(), in_maps,
                               core_ids=list(range(NCORES)))
    return _combine(out.results)


if __name__ == "__main__":
    rng = np.random.default_rng(0)
    ins = {
        "output": rng.standard_normal((B, C), dtype=np.float32),
        "target": rng.integers(0, C, size=(B,)),
        "master_net_pred": rng.standard_normal((B, C), dtype=np.float32),
        "feat_pooled": rng.standard_normal((B, D), dtype=np.float32),
        "feat_pooled_masked": rng.standard_normal((B, D), dtype=np.float32),
        "output_adv": rng.standard_normal((B, C), dtype=np.float32),
        "target_adv": rng.integers(0, C, size=(B,)),
    }
    print(kernel(**ins))


# revision 16
# speedup vs baseline: 1.7709x; 1.1805x over previous
"""Trainium2 Bass kernel for the combined loss -- symmetric-Gram variant.

Like kernel.py (see its docstring for the engine plan), but the InfoNCE
Gram exploits symmetry: with the per-core sample permutation
    order_c = concat(own(c), own(c+1), ..., own(c+7)),
    own(d)  = rows [d*512, d*512+512) of feat  ++  same rows of feat_masked,
each core computes exp-sums only for columns S0..S4 = [0, 5120):
  - S0 (own 1024x1024 block, self-diagonal masked) and S4 (the antipodal
    core's block, computed by both endpoints) contribute via row-accumulate
    only.
  - S1..S3 contribute row-accumulates AND per-column sums (ones-matmul on the
    exp tiles, accumulated across the 8 q-blocks in PSUM); by symmetry
    exp(s_ij)=exp(s_ji), so core c's column sums over S_k are exactly the
    missing row-sum pieces of core c+k's rows.  The host aligns and adds the
    gathered row/column partials, takes the log, and folds the NCE mean into
    the final scalar (the same gather+reduce arithmetic _combine always did).
Positive pairs live on the S0 block diagonal offset by 512; their exp is read
off the exp tile with an identity mask and logged on the host.
"""

import numpy as np

import concourse.bacc as bacc
import concourse.tile as tile
from concourse import mybir
from concourse.bass_utils import run_bass_kernel_spmd

F32 = mybir.dt.float32
BF16 = mybir.dt.bfloat16
AF = mybir.ActivationFunctionType
ALU = mybir.AluOpType
AX = mybir.AxisListType

NCORES = 8
B, C, D = 4096, 1000, 256
RB = B // NCORES          # 512 rows of the [B, C] tensors per core
NT = RB // 128            # 4 row-tiles per core
N2 = 2 * B                # 8192 infoNCE rows
N2C = 5120                # feature columns kept per core (blocks S0..S4)
CH = 2048                 # column chunk
SUB = 512                 # matmul moving free dim (PSUM bank limit)
QB = 8                    # 128-row q-blocks per core

KL_TEMP = 4.0
KL_INTERP = 0.5
NCE_TEMP = 0.07
NEG_BIG = -1.0e9


def _build_module():
    nc = bacc.Bacc("TRN2", target_bir_lowering=False, debug=False)

    o_d = nc.dram_tensor("o", [128, NT * C], BF16, kind="ExternalInput")
    m_d = nc.dram_tensor("m", [128, NT * C], BF16, kind="ExternalInput")
    a_d = nc.dram_tensor("a", [128, NT * C], BF16, kind="ExternalInput")
    tgta_d = nc.dram_tensor("tgta", [128, 2 * NT], F32, kind="ExternalInput")
    f0_d = nc.dram_tensor("f0", [128, N2C], BF16, kind="ExternalInput")
    f1_d = nc.dram_tensor("f1", [128, N2C], BF16, kind="ExternalInput")
    res_d = nc.dram_tensor("res", [8, 1], F32, kind="ExternalOutput")
    rs_d = nc.dram_tensor("rs", [128, QB], F32, kind="ExternalOutput")
    poss_d = nc.dram_tensor("poss", [128, QB], F32, kind="ExternalOutput")
    cs_d = nc.dram_tensor("cs", [1, 3072], F32, kind="ExternalOutput")

    import ml_dtypes
    cstf_np = np.concatenate(
        [np.tile(np.arange(C, dtype=np.float32), (128, 1)),
         np.eye(128, dtype=np.float32)], axis=1)
    cstb_np = np.concatenate(
        [np.eye(128), NEG_BIG * np.eye(128), np.ones((128, 128))],
        axis=1).astype(ml_dtypes.bfloat16)
    cstf_d = nc.inline_tensor(cstf_np, "cstf")
    cstb_d = nc.inline_tensor(cstb_np, "cstb")

    with tile.TileContext(nc) as tc:
        with (
            tc.tile_pool(name="persist", bufs=1) as persist,
            tc.tile_pool(name="scr", bufs=4) as scrp,
            tc.tile_pool(name="cescr", bufs=3) as cescr,
            tc.tile_pool(name="rnlp", bufs=3) as rnlp,
            tc.tile_pool(name="vec", bufs=1) as vecp,
        ):
            dma0 = nc.sync.dma_start
            psA_cm = tc.tile_pool(name="psA", bufs=2, space="PSUM")
            psA = psA_cm.__enter__()

            cstf_t = persist.tile([128, C + 128], F32, tag="cstf")
            cstb_t = persist.tile([128, 384], BF16, tag="cstb")
            tgta_t = persist.tile([128, 2 * NT], F32, tag="tgta")
            raw_pool_cm = tc.tile_pool(name="raw", bufs=1)
            rawp = raw_pool_cm.__enter__()
            h0 = rawp.tile([128, N2C], BF16, tag="h0")
            h1 = rawp.tile([128, N2C], BF16, tag="h1")
            h0b = persist.tile([128, N2C], BF16, tag="h0b")
            h1b = persist.tile([128, N2C], BF16, tag="h1b")
            o_t = persist.tile([128, NT * C], BF16, tag="o")
            m_t = persist.tile([128, NT * C], BF16, tag="m")
            a_t = persist.tile([128, NT * C], BF16, tag="a")

            iota_t = cstf_t[:, 0:C]
            ident_t = cstf_t[:, C:C + 128]
            identb_t = cstb_t[:, 0:128]
            negidb_t = cstb_t[:, 128:256]
            onesb_t = cstb_t[:, 256:384]
            tg_t = tgta_t[:, 0:NT]
            ta_t = tgta_t[:, NT:2 * NT]

            # norm chunks: [0,2048) [2048,4096) [4096,5120)
            NCHK = [(0, 2048), (2048, 4096), (4096, 5120)]

            dma0(out=h0[:, 0:2048], in_=f0_d[:, 0:2048])
            dma0(out=h1[:, 0:2048], in_=f1_d[:, 0:2048])
            dma0(out=cstb_t[:], in_=cstb_d[:])
            dma0(out=o_t[:], in_=o_d[:])
            dma0(out=h0[:, 2048:4096], in_=f0_d[:, 2048:4096])
            dma0(out=h1[:, 2048:4096], in_=f1_d[:, 2048:4096])
            dma0(out=tgta_t[:], in_=tgta_d[:])
            dma0(out=h0[:, 4096:5120], in_=f0_d[:, 4096:5120])
            dma0(out=h1[:, 4096:5120], in_=f1_d[:, 4096:5120])
            dma0(out=cstf_t[:], in_=cstf_d[:])
            dma0(out=m_t[:], in_=m_d[:])
            dma0(out=a_t[:], in_=a_d[:])

            # ---- stat tiles ----
            stats = vecp.tile([128, 16], F32, tag="stats")  # S1|ST|SM|SA
            gst = vecp.tile([128, 12], F32, tag="gst")      # GO | GA | PP
            rsP1 = vecp.tile([128, QB], F32, tag="rsP1")
            rs2 = vecp.tile([128, 3 * QB], F32, tag="rs2")
            poss = vecp.tile([128, QB], F32, tag="poss")

            # ---- norm phase: Ln group then Exp group ----
            rls = []
            for (lo, hi) in NCHK:
                w = hi - lo
                s0 = scrp.tile([128, w], BF16, tag="sq")
                nc.vector.tensor_mul(s0[:], h0[:, lo:hi], h0[:, lo:hi])
                s1 = scrp.tile([128, w], BF16, tag="sq")
                nc.vector.tensor_mul(s1[:], h1[:, lo:hi], h1[:, lo:hi])
                ps_n = psA.tile([128, CH], F32, tag="ps")
                for s in range(w // SUB):
                    ssl = slice(s * SUB, (s + 1) * SUB)
                    nc.tensor.matmul(ps_n[:, ssl], onesb_t, s0[:, ssl],
                                     start=True, stop=False)
                    nc.tensor.matmul(ps_n[:, ssl], onesb_t, s1[:, ssl],
                                     start=False, stop=True)
                rl = rnlp.tile([128, w], BF16, tag="rnl")
                nc.scalar.activation(rl[:], ps_n[:, 0:w], AF.Ln)
                rls.append(rl)

            for i, (lo, hi) in enumerate(NCHK):
                rnc = scrp.tile([128, hi - lo], BF16, tag="rnc")
                nc.scalar.activation(rnc[:], rls[i][:], AF.Exp, scale=-0.5)
                nc.vector.tensor_mul(h0b[:, lo:hi], h0[:, lo:hi], rnc[:])
                nc.vector.tensor_mul(h1b[:, lo:hi], h1[:, lo:hi], rnc[:])

            e1 = cescr.tile([128, NT * C], BF16, tag="ce")
            nc.scalar.activation(e1[:], o_t[:], AF.Exp, scale=1.0)
            eT = cescr.tile([128, NT * C], BF16, tag="ce")
            nc.scalar.activation(eT[:], o_t[:], AF.Exp,
                                 scale=float(1.0 / KL_TEMP))
            em = persist.tile([128, NT * C], BF16, tag="em")
            nc.scalar.activation(em[:], m_t[:], AF.Exp,
                                 scale=float(1.0 / KL_TEMP))
            ea = cescr.tile([128, NT * C], BF16, tag="ce")
            nc.scalar.activation(ea[:], a_t[:], AF.Exp, scale=1.0)

            raw_pool_cm.__exit__(None, None, None)
            esp_cm = tc.tile_pool(name="esp", bufs=6)
            esp = esp_cm.__enter__()

            # ---- cekl DVE chain ----
            for t in range(NT):
                ccl = slice(t * C, (t + 1) * C)
                nc.vector.reduce_sum(stats[:, t:t + 1], e1[:, ccl], axis=AX.X)
                nc.vector.reduce_sum(stats[:, 4 + t:5 + t], eT[:, ccl],
                                     axis=AX.X)
                nc.vector.reduce_sum(stats[:, 8 + t:9 + t], em[:, ccl],
                                     axis=AX.X)
            d_t = cescr.tile([128, NT * C], BF16, tag="d")
            nc.vector.tensor_sub(d_t[:], m_t[:], o_t[:])
            for t in range(NT):
                ccl = slice(t * C, (t + 1) * C)
                pr = scrp.tile([128, C], BF16, tag="pr")
                nc.vector.scalar_tensor_tensor(
                    out=pr[:], in0=d_t[:, ccl], scalar=1.0, in1=em[:, ccl],
                    op0=ALU.mult, op1=ALU.mult, accum_out=gst[:, 8 + t:9 + t])
                g1 = scrp.tile([128, C], BF16, tag="pr")
                nc.vector.scalar_tensor_tensor(
                    out=g1[:], in0=iota_t, scalar=tg_t[:, t:t + 1],
                    in1=o_t[:, ccl], op0=ALU.is_equal, op1=ALU.mult,
                    accum_out=gst[:, t:t + 1])
                g2 = scrp.tile([128, C], BF16, tag="pr")
                nc.vector.scalar_tensor_tensor(
                    out=g2[:], in0=iota_t, scalar=ta_t[:, t:t + 1],
                    in1=a_t[:, ccl], op0=ALU.is_equal, op1=ALU.mult,
                    accum_out=gst[:, 4 + t:5 + t])
                nc.vector.reduce_sum(stats[:, 12 + t:13 + t], ea[:, ccl],
                                     axis=AX.X)

            ptn = vecp.tile([128, NT], F32, tag="ptn")
            nc.scalar.activation(ptn[:], gst[:, 0:4], AF.Exp)

            # ---- InfoNCE pass 1: [S0 | S4] per q-block, row-accum only ----
            for qi in range(QB):
                q0 = 128 * qi
                lhsT0 = h0b[:, q0:q0 + 128]
                lhsT1 = h1b[:, q0:q0 + 128]
                ps_t = psA.tile([128, CH], F32, tag="ps")
                for s in range(4):
                    c0 = s * SUB if s < 2 else N2 // 2 + (s - 2) * SUB
                    ssl = slice(s * SUB, (s + 1) * SUB)
                    nc.tensor.matmul(ps_t[:, ssl], lhsT0,
                                     h0b[:, c0:c0 + SUB],
                                     start=True, stop=False)
                    if s == q0 // SUB:
                        nc.tensor.matmul(ps_t[:, q0:q0 + 128],
                                         negidb_t, identb_t,
                                         start=False, stop=False,
                                         skip_group_check=True)
                    nc.tensor.matmul(ps_t[:, ssl], lhsT1,
                                     h1b[:, c0:c0 + SUB],
                                     start=False, stop=True)
                es = esp.tile([128, CH], BF16, tag="es")
                nc.scalar.activation(es[:], ps_t[:], AF.Exp,
                                     scale=float(1.0 / NCE_TEMP),
                                     accum_out=rsP1[:, qi:qi + 1])
                poff = (q0 + 512) % 1024
                pscr = scrp.tile([128, 128], F32, tag="posscr")
                nc.vector.scalar_tensor_tensor(
                    out=pscr[:], in0=es[:, poff:poff + 128],
                    scalar=1.0, in1=ident_t,
                    op0=ALU.mult, op1=ALU.mult,
                    accum_out=poss[:, qi:qi + 1])

            # ---- InfoNCE pass 2: S1..S3, row-accum + column sums ----
            psA_cm.__exit__(None, None, None)
            gps_cm = tc.tile_pool(name="gps", bufs=2, space="PSUM")
            gps = gps_cm.__enter__()
            csps_cm = tc.tile_pool(name="csps", bufs=2, space="PSUM")
            csps = csps_cm.__enter__()

            for u in range(3):
                base = 1024 * (u + 1)
                cs_ps = csps.tile([128, 1024], F32, tag="cs")
                for qi in range(QB):
                    q0 = 128 * qi
                    lhsT0 = h0b[:, q0:q0 + 128]
                    lhsT1 = h1b[:, q0:q0 + 128]
                    g = gps.tile([128, 1024], F32, tag="g")
                    for s in range(2):
                        c0 = base + s * SUB
                        ssl = slice(s * SUB, (s + 1) * SUB)
                        nc.tensor.matmul(g[:, ssl], lhsT0,
                                         h0b[:, c0:c0 + SUB],
                                         start=True, stop=False)
                        nc.tensor.matmul(g[:, ssl], lhsT1,
                                         h1b[:, c0:c0 + SUB],
                                         start=False, stop=True)
                    es2 = esp.tile([128, 1024], BF16, tag="es")
                    nc.scalar.activation(es2[:], g[:], AF.Exp,
                                         scale=float(1.0 / NCE_TEMP),
                                         accum_out=rs2[:, 3 * qi + u:
                                                       3 * qi + u + 1])
                    for s in range(2):
                        ssl = slice(s * SUB, (s + 1) * SUB)
                        nc.tensor.matmul(cs_ps[:, ssl], onesb_t,
                                         es2[:, ssl],
                                         start=(qi == 0), stop=(qi == QB - 1),
                                         skip_group_check=True)
                cs_sb = scrp.tile([128, 1024], F32, tag="cssb")
                nc.vector.tensor_copy(cs_sb[:], cs_ps[:])
                dma0(out=cs_d[:, 1024 * u:1024 * (u + 1)], in_=cs_sb[0:1, :])

            # per-q row-sum totals -> [128, 8] out
            rtmp = vecp.tile([128, QB], F32, tag="rtmp")
            for qi in range(QB):
                nc.vector.reduce_sum(rtmp[:, qi:qi + 1],
                                     rs2[:, 3 * qi:3 * qi + 3], axis=AX.X)
            rsout = vecp.tile([128, QB], F32, tag="rsout")
            nc.vector.tensor_add(rsout[:], rtmp[:], rsP1[:])
            dma0(out=rs_d[:], in_=rsout[:])
            dma0(out=poss_d[:], in_=poss[:])

            # ---- epilogue: one packed Ln, rest on DVE ----
            lall = vecp.tile([128, 16], F32, tag="lall")
            nc.scalar.activation(lall[:], stats[:], AF.Ln)

            ce = vecp.tile([128, NT], F32, tag="ce4")
            nc.vector.tensor_sub(ce[:], lall[:, 0:4], gst[:, 0:4])
            adv = vecp.tile([128, NT], F32, tag="adv")
            nc.vector.tensor_sub(adv[:], lall[:, 12:16], gst[:, 4:8])

            invSM = vecp.tile([128, NT], F32, tag="invSM")
            nc.vector.reciprocal(invSM[:], stats[:, 8:12])
            kl = vecp.tile([128, NT], F32, tag="kl")
            nc.vector.tensor_mul(kl[:], gst[:, 8:12], invSM[:])
            nc.vector.tensor_scalar_mul(kl[:], kl[:], float(1.0 / KL_TEMP))
            nc.vector.tensor_sub(kl[:], kl[:], lall[:, 8:12])
            nc.vector.tensor_add(kl[:], kl[:], lall[:, 4:8])

            pt = vecp.tile([128, NT], F32, tag="pt")
            invS1 = vecp.tile([128, NT], F32, tag="invS1")
            nc.vector.reciprocal(invS1[:], stats[:, 0:4])
            nc.vector.tensor_mul(pt[:], ptn[:], invS1[:])
            c1 = vecp.tile([128, NT], F32, tag="c1")
            nc.vector.tensor_scalar(c1[:], pt[:], 0.5, None, op0=ALU.is_lt)
            c2 = vecp.tile([128, NT], F32, tag="c2")
            nc.vector.tensor_scalar(c2[:], pt[:], 0.2, None, op0=ALU.is_lt)
            u_ = vecp.tile([128, NT], F32, tag="u")
            nc.vector.tensor_scalar(u_[:], pt[:], -1.0, 1.0,
                                    op0=ALU.mult, op1=ALU.add)
            u2 = vecp.tile([128, NT], F32, tag="u2")
            nc.vector.tensor_mul(u2[:], u_[:], u_[:])
            u3 = vecp.tile([128, NT], F32, tag="u3")
            nc.vector.tensor_mul(u3[:], u2[:], u_[:])
            u5 = vecp.tile([128, NT], F32, tag="u5")
            nc.vector.tensor_mul(u5[:], u2[:], u3[:])
            ta_3 = vecp.tile([128, NT], F32, tag="ta3")
            nc.vector.tensor_sub(ta_3[:], u3[:], u_[:])
            tb_5 = vecp.tile([128, NT], F32, tag="tb5")
            nc.vector.tensor_sub(tb_5[:], u5[:], u3[:])
            w = vecp.tile([128, NT], F32, tag="w")
            nc.vector.tensor_mul(w[:], c1[:], ta_3[:])
            nc.vector.tensor_add(w[:], w[:], u_[:])
            wb = vecp.tile([128, NT], F32, tag="wb")
            nc.vector.tensor_mul(wb[:], c2[:], tb_5[:])
            nc.vector.tensor_add(w[:], w[:], wb[:])
            foc = vecp.tile([128, NT], F32, tag="foc")
            nc.vector.tensor_mul(foc[:], w[:], ce[:])

            acc = vecp.tile([128, 8], F32, tag="acc")
            nc.vector.reduce_sum(acc[:, 0:1], kl[:], axis=AX.X)
            nc.vector.reduce_sum(acc[:, 1:2], ce[:], axis=AX.X)
            nc.vector.reduce_sum(acc[:, 2:3], adv[:], axis=AX.X)
            nc.vector.reduce_sum(acc[:, 3:4], foc[:], axis=AX.X)
            nc.vector.memset(acc[:, 4:8], 0.0)

            onesf = vecp.tile([128, 1], F32, tag="onesf")
            nc.vector.memset(onesf[:], 1.0)
            ps_f = gps.tile([8, 1], F32, tag="g")
            nc.tensor.matmul(ps_f[:], acc[:], onesf[:],
                             start=True, stop=True)
            out_sb = vecp.tile([8, 1], F32, tag="out_sb")
            nc.scalar.copy(out_sb[:], ps_f[:])
            dma0(out=res_d[:], in_=out_sb[:])

            esp_cm.__exit__(None, None, None)
            csps_cm.__exit__(None, None, None)
            gps_cm.__exit__(None, None, None)

    nc.compile()
    return nc


_NC = None


def _get_nc():
    global _NC
    if _NC is None:
        _NC = _build_module()
    return _NC


def _own(d):
    return np.concatenate([np.arange(d * RB, (d + 1) * RB),
                           B + np.arange(d * RB, (d + 1) * RB)])


def _prep_inputs(output, target, master_net_pred, feat_pooled,
                 feat_pooled_masked, output_adv, target_adv):
    import ml_dtypes
    bf16 = ml_dtypes.bfloat16
    o = np.asarray(output, dtype=np.float32)
    m = np.asarray(master_net_pred, dtype=np.float32)
    a = np.asarray(output_adv, dtype=np.float32)
    tg = np.asarray(target).astype(np.int64)
    ta = np.asarray(target_adv).astype(np.int64)
    f0 = np.asarray(feat_pooled, dtype=np.float32)
    f1 = np.asarray(feat_pooled_masked, dtype=np.float32)
    feats = np.concatenate([f0, f1], axis=0)  # [2B, D]

    def pack(x, sl):
        return np.ascontiguousarray(
            x[sl].reshape(NT, 128, C).transpose(1, 0, 2).reshape(128, NT * C)
            .astype(bf16))

    in_maps = []
    for cc in range(NCORES):
        sl = slice(cc * RB, (cc + 1) * RB)
        order5 = np.concatenate([_own((cc + k) % 8) for k in range(5)])
        ftc = feats[order5].T.astype(bf16)  # [D, 5120]
        tgta = np.concatenate([tg[sl].reshape(NT, 128).T,
                               ta[sl].reshape(NT, 128).T],
                              axis=1).astype(np.float32)
        in_maps.append({
            "o": pack(o, sl),
            "m": pack(m, sl),
            "a": pack(a, sl),
            "tgta": np.ascontiguousarray(tgta),
            "f0": np.ascontiguousarray(ftc[0:128]),
            "f1": np.ascontiguousarray(ftc[128:256]),
        })
    return in_maps


def _combine(results):
    r = np.zeros(8, dtype=np.float64)
    for rr in results:
        r += rr["res"].reshape(-1).astype(np.float64)
    kl_mean = r[0] / (B * C)
    ce_mean = r[1] / B
    adv_mean = r[2] / B
    foc_mean = r[3] / B

    # NCE: align own row-sums with the symmetric column-sum partials from the
    # three cores whose strips cover this core's rows, then log + mean.
    R = [rr["rs"].T.reshape(-1).astype(np.float64) for rr in results]
    P = [rr["poss"].T.reshape(-1).astype(np.float64) for rr in results]
    CS = [rr["cs"].reshape(-1).astype(np.float64) for rr in results]
    nce_sum = 0.0
    for c in range(NCORES):
        total = (R[c]
                 + CS[(c + 7) % 8][0:1024]
                 + CS[(c + 6) % 8][1024:2048]
                 + CS[(c + 5) % 8][2048:3072])
        nce_sum += float(np.sum(np.log(total) - np.log(P[c])))
    nce_mean = nce_sum / N2

    loss = (KL_INTERP * KL_TEMP * KL_TEMP) * kl_mean \
        + (1.0 - KL_INTERP) * ce_mean + nce_mean + foc_mean + adv_mean
    return np.asarray([loss], dtype=np.float32)


def kernel(**inputs):
    in_maps = _prep_inputs(**inputs)
    out = run_bass_kernel_spmd(_get_nc(), in_maps,
                               core_ids=list(range(NCORES)))
    return _combine(out.results)


if __name__ == "__main__":
    rng = np.random.default_rng(0)
    ins = {
        "output": rng.standard_normal((B, C), dtype=np.float32),
        "target": rng.integers(0, C, size=(B,)),
        "master_net_pred": rng.standard_normal((B, C), dtype=np.float32),
        "feat_pooled": rng.standard_normal((B, D), dtype=np.float32),
        "feat_pooled_masked": rng.standard_normal((B, D), dtype=np.float32),
        "output_adv": rng.standard_normal((B, C), dtype=np.float32),
        "target_adv": rng.integers(0, C, size=(B,)),
    }
    print(kernel(**ins))
